# revision 9
# baseline (speedup 1.0000x reference)
"""Transformer-XL style MultiHeadAttention on 8 TRN2 NeuronCores.

Data-parallel over batch N=16 -> 2 batches per core. Full computation on
device per core:
  - per-head projections q/k/v (shared 64x64 weights, block-diag over head
    pairs on the 128-wide PE contraction)
  - relative key table rel = pos_emb @ Wr.T computed on device (pos_emb is an
    input-independent sinusoid constant, host-precomputed, fed reversed and
    transposed so the Toeplitz shift becomes a forward-strided read)
  - energy = (q+u)k^T + (q+v)rel^T(shifted) + mask bias, softmax, attn@v,
    output projection with Wo/bo
  - the relative-position shift (energy[q,k] takes rel index q-k+512) is done
    by a DRAM roundtrip: P[128,4,640](bf16) -> one diagonal-strided 3D DMA
    read (row stride 639 elements) -> [128,4,512] aligned tiles ->
    identity-matmul accumulate into the content PSUM.

Matmuls run in bf16 (f32 PSUM accumulation); softmax in f32 on ACT with
fused row-sum (accum_out). Input/weight transposes run on the PE
(transpose mode); the attention transpose for attn@v reuses the attention
DRAM output via xbar DMA-transpose loads.
"""

import numpy as np
import ml_dtypes

import concourse.bass as bass
import concourse.mybir as mybir
import concourse.tile as tile
from concourse import bacc
from concourse.bass_utils import run_bass_kernel_spmd

F32 = mybir.dt.float32
BF16 = mybir.dt.bfloat16
I32 = mybir.dt.int32

EMBED = 512
HEADS = 8
HEAD_D = 64
NB = 16          # full batch
NL = 2           # batches per core
QL = 512
KL = 512
R = QL + KL      # 1024 relative positions
NCORES = 8
HP = HEADS // 2  # head pairs

LAST_RESULTS = None
USE_POOL_NORM = True
AF = mybir.ActivationFunctionType
ALU = mybir.AluOpType


def build_program(masked=True):
    nc = bacc.Bacc("TRN2", target_bir_lowering=False, debug=False)

    # ---- I/O ----
    q_in = nc.dram_tensor("q_in", [NL, QL, EMBED], F32, kind="ExternalInput")
    k_in = nc.dram_tensor("k_in", [NL, KL, EMBED], F32, kind="ExternalInput")
    v_in = nc.dram_tensor("v_in", [NL, KL, EMBED], F32, kind="ExternalInput")
    m_in = (nc.dram_tensor("m_in", [NL, QL, KL], I32, kind="ExternalInput")
            if masked else None)
    wq_in = nc.dram_tensor("wq_in", [HEAD_D, HEAD_D], F32, kind="ExternalInput")
    wk_in = nc.dram_tensor("wk_in", [HEAD_D, HEAD_D], F32, kind="ExternalInput")
    wv_in = nc.dram_tensor("wv_in", [HEAD_D, HEAD_D], F32, kind="ExternalInput")
    wr_in = nc.dram_tensor("wr_in", [EMBED, EMBED], F32, kind="ExternalInput")
    wo_in = nc.dram_tensor("wo_in", [EMBED, EMBED], F32, kind="ExternalInput")
    u_in = nc.dram_tensor("u_in", [1, EMBED], F32, kind="ExternalInput")
    vb_in = nc.dram_tensor("vb_in", [1, EMBED], F32, kind="ExternalInput")
    bo_in = nc.dram_tensor("bo_in", [1, EMBED], F32, kind="ExternalInput")
    post_in = nc.dram_tensor("post_in", [EMBED, R], BF16, kind="ExternalInput")
    id_in = nc.dram_tensor("id_in", [128, 128], BF16, kind="ExternalInput")
    onescol_in = nc.dram_tensor("onescol_in", [1, 128], BF16, kind="ExternalInput")

    attn_o = nc.dram_tensor("attn_o", [NL, HEADS, QL, KL], BF16,
                            kind="ExternalOutput")
    out_o = nc.dram_tensor("out_o", [NL, QL, EMBED], F32, kind="ExternalOutput")

    with tile.TileContext(nc) as tc:
        with tc.tile_pool(name="persist", bufs=1) as persist, \
             tc.tile_pool(name="slab", bufs=1) as slab, \
             tc.tile_pool(name="proj", bufs=2) as proj, \
             tc.tile_pool(name="hdat", bufs=2) as hdat, \
             tc.tile_pool(name="setuppool", bufs=2) as setuppool, \
             tc.tile_pool(name="small", bufs=4) as small, \
             tc.tile_pool(name="pdpool", bufs=4) as pdpool, \
             tc.tile_pool(name="pospool", bufs=5) as pospool, \
             tc.tile_pool(name="outp", bufs=2) as outp, \
             tc.tile_pool(name="pp", bufs=2, space="PSUM") as pp, \
             tc.tile_pool(name="cc", bufs=2, space="PSUM") as cc, \
             tc.tile_pool(name="mm", bufs=2, space="PSUM") as mm, \
             tc.tile_pool(name="dscratch", bufs=5, space="DRAM") as dscratch:

            # ================= SETUP =================
            ident = persist.tile([128, 128], BF16)
            nc.sync.dma_start(out=ident[:], in_=id_in[:])
            onescol = persist.tile([1, 128], BF16)
            nc.sync.dma_start(out=onescol[:], in_=onescol_in[:])

            post = persist.tile([128, 4, R], BF16)  # [e within tile, etile, r]
            nc.sync.dma_start(
                out=post[:],
                in_=post_in[:].rearrange("(et e) r -> e et r", e=128))

            # per-partition bias vectors for head pairs: [128, HP]
            u_sb = persist.tile([128, HP], F32)
            vb_sb = persist.tile([128, HP], F32)
            for hp in range(HP):
                nc.sync.dma_start(out=u_sb[:, hp:hp + 1],
                                  in_=u_in[0, 128 * hp:128 * hp + 128])
                nc.sync.dma_start(out=vb_sb[:, hp:hp + 1],
                                  in_=vb_in[0, 128 * hp:128 * hp + 128])
            bo_b = persist.tile([1, EMBED], BF16)
            bo_f = setuppool.tile([1, EMBED], F32, tag="bof")
            nc.sync.dma_start(out=bo_f[:], in_=bo_in[:])
            nc.vector.tensor_copy(bo_b[:], bo_f[:])

            # f32 identity for PE transposes of f32 data
            idf = persist.tile([128, 128], F32)
            nc.vector.tensor_copy(idf[:], ident[:])

            # small weight transposes via PE transpose-mode
            bds = {}
            for name, w_dram in (("q", wq_in), ("k", wk_in), ("v", wv_in)):
                w_f = setuppool.tile([64, 64], F32, tag="w_f")
                nc.sync.dma_start(out=w_f[:], in_=w_dram[:])
                wtp = mm.tile([64, 64], F32, tag="mmt")
                nc.tensor.transpose(wtp[:], w_f[:], idf[0:64, 0:64])
                bd = persist.tile([128, 128], BF16, tag="bd" + name)
                nc.vector.memset(bd[:], 0.0)
                nc.vector.tensor_copy(bd[0:64, 0:64], wtp[:])
                nc.vector.tensor_copy(bd[64:128, 64:128], wtp[:])
                bds[name] = bd

            # Wr^T, Wo^T via PE transpose: layout [e(part), etile, e']
            wrt = persist.tile([128, 4, EMBED], BF16, tag="wrt")
            wot = persist.tile([128, 4, EMBED], BF16, tag="wot")
            for w_dram, dst in ((wr_in, wrt), (wo_in, wot)):
                for ot in range(4):  # e' tile (partition rows of source)
                    wrow_f = setuppool.tile([128, EMBED], F32, tag="wrow_f")
                    nc.sync.dma_start(out=wrow_f[:],
                                      in_=w_dram[128 * ot:128 * ot + 128, :])
                    wtp = mm.tile([128, 512], F32, tag="mmt")
                    for it in range(4):
                        nc.tensor.transpose(wtp[:, 128 * it:128 * it + 128],
                                            wrow_f[:, 128 * it:128 * it + 128],
                                            idf[:])
                    for it in range(4):
                        nc.scalar.activation(dst[:, it, 128 * ot:128 * ot + 128],
                                             wtp[:, 128 * it:128 * it + 128],
                                             AF.Identity)

            # rel table: rel_sb[e'g, e't, jcol] = rel[1023-jcol, e'] (bf16)
            rel_sb = persist.tile([128, 4, R], BF16, tag="rel")
            for et in range(4):
                for rh in range(2):
                    rpsum = mm.tile([128, 512], F32, tag="mmt")
                    for e in range(4):
                        nc.tensor.matmul(
                            rpsum[:],
                            wrt[:, e, 128 * et:128 * et + 128],
                            post[:, e, 512 * rh:512 * rh + 512],
                            start=(e == 0), stop=(e == 3))
                    nc.scalar.activation(
                        rel_sb[:, et, 512 * rh:512 * rh + 512], rpsum[:],
                        AF.Identity)

            # mask bias: (mask-1)*1e20 as bf16, [128, NL*4, 512]
            maskb = None
            if masked:
                maskb = persist.tile([128, NL * 4, 512], BF16, tag="maskb")
                for n in range(NL):
                    mrow = setuppool.tile([128, 4, 512], I32, tag="mrow")
                    nc.sync.dma_start(
                        out=mrow[:],
                        in_=m_in[n].rearrange("(qt p) k -> p qt k", p=128))
                    for qt in range(4):
                        nc.vector.tensor_scalar(
                            out=maskb[:, 4 * n + qt, :], in0=mrow[:, qt, :],
                            scalar1=-1.0, scalar2=1.0e20,
                            op0=ALU.add, op1=ALU.mult)

            # ============ per-batch state build ============
            def emit_slabs(n):
                """load x f32, SWDGE cast-write bf16 to DRAM, xbar-load
                transposed -> xt dict [e, etile, tok]"""
                out = {}
                for name, src in (("q", q_in), ("k", k_in), ("v", v_in)):
                    x_f = slab.tile([128, 4, EMBED], F32, tag="x_f")
                    nc.sync.dma_start(
                        out=x_f[:],
                        in_=src[n].rearrange("(tt t) e -> t tt e", t=128))
                    xscr = dscratch.tile([QL, EMBED], BF16, tag="xscr")
                    nc.gpsimd.dma_start(
                        out=xscr[:].rearrange("(tt p) e -> p tt e", p=128),
                        in_=x_f[:])
                    xt = slab.tile([128, 4, QL], BF16, tag="xt_" + name)
                    for et in range(4):
                        nc.scalar.dma_start_transpose(
                            out=xt[:, et, :],
                            in_=xscr[:, 128 * et:128 * et + 128])
                    out[name] = xt
                return out

            def emit_proj(n, xt):
                """projections for batch n -> (qu, qv, kt, v) tiles"""
                qu = proj.tile([128, HP, QL], BF16, tag="qu")
                qv = proj.tile([128, HP, QL], BF16, tag="qv")
                kt = proj.tile([128, HP, KL], BF16, tag="kt")
                vsb = proj.tile([128, 4, EMBED], BF16, tag="vsb")
                for hp in range(HP):
                    qp = mm.tile([128, QL], F32, tag="mmt")
                    nc.tensor.matmul(qp[:], bds["q"], xt["q"][:, hp, :],
                                     start=True, stop=True)
                    nc.vector.tensor_scalar(out=qu[:, hp, :], in0=qp[:],
                                            scalar1=u_sb[:, hp:hp + 1],
                                            scalar2=None, op0=ALU.add)
                    nc.vector.tensor_scalar(out=qv[:, hp, :], in0=qp[:],
                                            scalar1=vb_sb[:, hp:hp + 1],
                                            scalar2=None, op0=ALU.add)
                    kp = mm.tile([128, KL], F32, tag="mmt")
                    nc.tensor.matmul(kp[:], bds["k"], xt["k"][:, hp, :],
                                     start=True, stop=True)
                    nc.scalar.activation(kt[:, hp, :], kp[:], AF.Identity)
                    vp = mm.tile([128, 512], F32, tag="mmt")
                    for tt in range(4):
                        nc.tensor.matmul(vp[:, 128 * tt:128 * tt + 128],
                                         xt["v"][:, hp, 128 * tt:128 * tt + 128],
                                         bds["v"], start=True, stop=True)
                    for tt in range(4):
                        nc.scalar.activation(
                            vsb[:, tt, 128 * hp:128 * hp + 128],
                            vp[:, 128 * tt:128 * tt + 128], AF.Identity)
                return qu, qv, kt, vsb

            # ============ streamed attention ============
            def head_slice(t, h, qt=None):
                base = (h % 2) * 64
                if qt is None:
                    return t[base:base + 64, h // 2, :]
                return t[base:base + 64, h // 2, 128 * qt:128 * qt + 128]

            def emit_P(n, h, st):
                """position matmuls + drains + scratch write + diag read"""
                qv = st["proj"][1]
                rel_h = rel_sb[(h % 2) * 64:(h % 2) * 64 + 64, h // 2, :]
                pd4 = pdpool.tile([128, 4, 640], BF16, tag="pd4")
                for qt in range(4):
                    ws = 384 - 128 * qt
                    P = pp.tile([128, 640], F32, tag="pp")
                    nc.tensor.matmul(P[:, 0:512], head_slice(qv, h, qt),
                                     rel_h[:, ws:ws + 512], start=True, stop=True)
                    nc.tensor.matmul(P[:, 512:640], head_slice(qv, h, qt),
                                     rel_h[:, ws + 512:ws + 640],
                                     start=True, stop=True)
                    nc.vector.tensor_copy(pd4[:, qt, :], P[:])
                scr = dscratch.tile([4, 128, 640], BF16, tag="scr")
                # write [p, qt, j] -> [qt, p, j]
                nc.gpsimd.dma_start(
                    out=scr[:].rearrange("qt p j -> p qt j"), in_=pd4[:])
                # diagonal read: pos[p, qt, k] = scr[qt, p, 127 - p + k]
                pos4 = pospool.tile([128, 4, 512], BF16, tag="pos4")
                dsrc = bass.AP(scr[:].tensor, 127,
                               [[639, 128], [128 * 640, 4], [1, 512]])
                nc.gpsimd.dma_start(out=pos4[:], in_=dsrc)
                st["pos4"] = pos4

            def emit_C(n, h, st):
                qu, _, kt, _ = st["proj"]
                st["Z"] = small.tile([128, 4], F32, tag="Z", name="zt")
                st["expm"] = hdat.tile([128, 4, 512], BF16, tag="expm",
                                       name="expm")
                for qt in range(4):
                    C = cc.tile([128, 512], F32, tag="cc")
                    nc.tensor.matmul(C[:], head_slice(qu, h, qt),
                                     kt[(h % 2) * 64:(h % 2) * 64 + 64, h // 2, :],
                                     start=True, stop=False)
                    if masked:
                        nc.tensor.matmul(C[:], ident[:], maskb[:, 4 * n + qt, :],
                                         start=False, stop=False)
                    nc.tensor.matmul(C[:], ident[:], st["pos4"][:, qt, :],
                                     start=False, stop=True)
                    nc.scalar.activation(st["expm"][:, qt, :], C[:], AF.Exp,
                                         scale=0.125,
                                         accum_out=st["Z"][:, qt:qt + 1])

            def emit_F(n, h, st):
                rc = small.tile([128, 4], F32, tag="rc")
                nc.vector.reciprocal(rc[:], st["Z"][:])
                attn_n = hdat.tile([128, 4, 512], BF16, tag="attn_n")
                for qt in range(4):
                    norm_eng = nc.gpsimd if USE_POOL_NORM else nc.vector
                    norm_eng.tensor_scalar(
                        out=attn_n[:, qt, :], in0=st["expm"][:, qt, :],
                        scalar1=rc[:, qt:qt + 1], scalar2=None, op0=ALU.mult)
                # one batched output write [p, qt, k] -> attn[(qt p), k]
                nc.sync.dma_start(
                    out=attn_o[n, h].rearrange("(qt p) k -> p qt k", p=128),
                    in_=attn_n[:])
                # transposed reload from DRAM via xbar (ACT hwdge ring)
                attn_t = hdat.tile([128, 4, 512], BF16, tag="attn_t")
                for ktile in range(4):
                    eng = nc.scalar
                    eng.dma_start_transpose(
                        out=attn_t[:, ktile, :],
                        in_=attn_o[n, h][:, 128 * ktile:128 * ktile + 128])
                # ctx^T[d, q] = sum_k v[k, d] attn_t[k, q]
                vsb = st["proj"][3]
                ctxp = mm.tile([64, 512], F32, tag="mmt")
                for ktile in range(4):
                    nc.tensor.matmul(
                        ctxp[:],
                        vsb[:, ktile, 64 * h:64 * h + 64],
                        attn_t[:, ktile, :],
                        start=(ktile == 0), stop=(ktile == 3))
                nc.scalar.activation(
                    st["ctx"][(h % 2) * 64:(h % 2) * 64 + 64, h // 2, :],
                    ctxp[:], AF.Identity)

            def emit_E(n, st):
                osb = outp.tile([128, 4, 512], F32, tag="osb")
                for tt in range(4):
                    op = mm.tile([128, 512], F32, tag="mmt")
                    for et in range(4):
                        nc.tensor.matmul(
                            op[:], st["ctx"][:, et, 128 * tt:128 * tt + 128],
                            wot[:, et, :], start=(et == 0), stop=False)
                    nc.tensor.matmul(op[:], onescol[:], bo_b[:],
                                     start=False, stop=True)
                    nc.vector.tensor_copy(osb[:, tt, :], op[:])
                nc.sync.dma_start(
                    out=out_o[n].rearrange("(tt p) e -> p tt e", p=128),
                    in_=osb[:])

            # ---- main stream over heads with software pipelining ----
            SKEW = 4  # heads of lookahead between P and C phases
            heads = [(n, h) for n in range(NL) for h in range(HEADS)]
            state = {}
            fqueue = []
            cur_n = -1
            for i, (n, h) in enumerate(heads):
                if n != cur_n:
                    cur_n = n
                    xt = emit_slabs(n)
                    projt = emit_proj(n, xt)
                    ctx_sb = proj.tile([128, HP, QL], BF16, tag="ctx",
                                       name="ctx_sb")
                    nstate = {"proj": projt, "ctx": ctx_sb}
                state[(n, h)] = dict(nstate)
                emit_P(n, h, state[(n, h)])
                j = i - SKEW
                if j >= 0:
                    jn, jh = heads[j]
                    emit_C(jn, jh, state[(jn, jh)])
                    fqueue.append((jn, jh))
                    if len(fqueue) > 1:
                        fn, fh = fqueue.pop(0)
                        emit_F(fn, fh, state[(fn, fh)])
                        if fh == HEADS - 1:
                            emit_E(fn, state[(fn, fh)])
            for j in range(len(heads) - SKEW, len(heads)):
                jn, jh = heads[j]
                emit_C(jn, jh, state[(jn, jh)])
                fqueue.append((jn, jh))
            while fqueue:
                fn, fh = fqueue.pop(0)
                emit_F(fn, fh, state[(fn, fh)])
                if fh == HEADS - 1:
                    emit_E(fn, state[(fn, fh)])

    nc.compile()
    return nc


def _host_constants():
    invf = (10000.0 ** (-np.arange(0, EMBED, 2, dtype=np.float64) / EMBED))
    seq = np.arange(R, dtype=np.float64)[:, None] * invf[None, :]
    pos_emb = np.concatenate([np.sin(seq), np.cos(seq)], axis=-1)  # [R, E]
    post_rev = np.ascontiguousarray(pos_emb[::-1, :].T).astype(ml_dtypes.bfloat16)
    ident = np.eye(128, dtype=ml_dtypes.bfloat16)
    onescol = np.ones((1, 128), dtype=ml_dtypes.bfloat16)
    return post_rev, ident, onescol


def kernel(values, keys, query, mask, Wv, Wk, Wq, Wr, u_bias, v_bias, Wo, bo):
    values = np.ascontiguousarray(np.asarray(values, dtype=np.float32))
    keys = np.ascontiguousarray(np.asarray(keys, dtype=np.float32))
    query = np.ascontiguousarray(np.asarray(query, dtype=np.float32))
    mask = np.ascontiguousarray(np.asarray(mask, dtype=np.int32))
    Wv = np.ascontiguousarray(np.asarray(Wv, dtype=np.float32))
    Wk = np.ascontiguousarray(np.asarray(Wk, dtype=np.float32))
    Wq = np.ascontiguousarray(np.asarray(Wq, dtype=np.float32))
    Wr = np.ascontiguousarray(np.asarray(Wr, dtype=np.float32))
    Wo = np.ascontiguousarray(np.asarray(Wo, dtype=np.float32))
    u_flat = np.ascontiguousarray(np.asarray(u_bias, dtype=np.float32).reshape(1, EMBED))
    v_flat = np.ascontiguousarray(np.asarray(v_bias, dtype=np.float32).reshape(1, EMBED))
    bo_flat = np.ascontiguousarray(np.asarray(bo, dtype=np.float32).reshape(1, EMBED))

    post_rev, ident, onescol = _host_constants()

    masked = bool((mask == 0).any())
    nc = build_program(masked=masked)

    in_maps = []
    for c in range(NCORES):
        s = slice(NL * c, NL * (c + 1))
        in_maps.append({
            "q_in": query[s], "k_in": keys[s], "v_in": values[s],
            **({"m_in": mask[s]} if masked else {}),
            "wq_in": Wq, "wk_in": Wk, "wv_in": Wv, "wr_in": Wr, "wo_in": Wo,
            "u_in": u_flat, "vb_in": v_flat, "bo_in": bo_flat,
            "post_in": post_rev, "id_in": ident, "onescol_in": onescol,
        })

    res = run_bass_kernel_spmd(nc, in_maps, core_ids=list(range(NCORES)))
    global LAST_RESULTS
    LAST_RESULTS = res

    out = np.empty((NB, QL, EMBED), dtype=np.float32)
    attn = np.empty((NB, HEADS, QL, KL), dtype=np.float32)
    for c in range(NCORES):
        r = res.results[c]
        out[NL * c:NL * (c + 1)] = r["out_o"]
        attn[NL * c:NL * (c + 1)] = np.asarray(r["attn_o"], dtype=np.float32)
    return out, attn


# revision 10
# speedup vs baseline: 1.8528x; 1.8528x over previous
"""Transformer-XL style MultiHeadAttention on 8 TRN2 NeuronCores.

Data-parallel over batch N=16 -> 2 batches per core. Full computation on
device per core:
  - per-head projections q/k/v (shared 64x64 weights, block-diag over head
    pairs on the 128-wide PE contraction)
  - relative key table rel = pos_emb @ Wr.T computed on device (pos_emb is an
    input-independent sinusoid constant, host-precomputed, fed reversed and
    transposed so the Toeplitz shift becomes a forward-strided read)
  - energy = (q+u)k^T + (q+v)rel^T(shifted) + mask bias, softmax, attn@v,
    output projection with Wo/bo
  - the relative-position shift (energy[q,k] takes rel index q-k+512) is done
    by a DRAM roundtrip: P[128,4,640](bf16) -> one diagonal-strided 3D DMA
    read (row stride 639 elements) -> [128,4,512] aligned tiles ->
    identity-matmul accumulate into the content PSUM.

Matmuls run in bf16 (f32 PSUM accumulation); softmax in f32 on ACT with
fused row-sum (accum_out). Input/weight transposes run on the PE
(transpose mode); the attention transpose for attn@v reuses the attention
DRAM output via xbar DMA-transpose loads.
"""

import numpy as np
import ml_dtypes

import concourse.bass as bass
import concourse.mybir as mybir
import concourse.tile as tile
from concourse import bacc
from concourse.bass_utils import run_bass_kernel_spmd

F32 = mybir.dt.float32
BF16 = mybir.dt.bfloat16
I32 = mybir.dt.int32

EMBED = 512
HEADS = 8
HEAD_D = 64
NB = 16          # full batch
NL = 2           # batches per core
QL = 512
KL = 512
R = QL + KL      # 1024 relative positions
NCORES = 8
HP = HEADS // 2  # head pairs

LAST_RESULTS = None
USE_POOL_NORM = False
AF = mybir.ActivationFunctionType
ALU = mybir.AluOpType


def build_program(masked=True):
    nc = bacc.Bacc("TRN2", target_bir_lowering=False, debug=False)

    # ---- I/O ----
    q_in = nc.dram_tensor("q_in", [NL, QL, EMBED], F32, kind="ExternalInput")
    k_in = nc.dram_tensor("k_in", [NL, KL, EMBED], F32, kind="ExternalInput")
    v_in = nc.dram_tensor("v_in", [NL, KL, EMBED], F32, kind="ExternalInput")
    m_in = (nc.dram_tensor("m_in", [NL, QL, KL], I32, kind="ExternalInput")
            if masked else None)
    wq_in = nc.dram_tensor("wq_in", [HEAD_D, HEAD_D], F32, kind="ExternalInput")
    wk_in = nc.dram_tensor("wk_in", [HEAD_D, HEAD_D], F32, kind="ExternalInput")
    wv_in = nc.dram_tensor("wv_in", [HEAD_D, HEAD_D], F32, kind="ExternalInput")
    wr_in = nc.dram_tensor("wr_in", [EMBED, EMBED], F32, kind="ExternalInput")
    wo_in = nc.dram_tensor("wo_in", [EMBED, EMBED], F32, kind="ExternalInput")
    u_in = nc.dram_tensor("u_in", [1, EMBED], F32, kind="ExternalInput")
    vb_in = nc.dram_tensor("vb_in", [1, EMBED], F32, kind="ExternalInput")
    bo_in = nc.dram_tensor("bo_in", [1, EMBED], F32, kind="ExternalInput")
    post_in = nc.dram_tensor("post_in", [EMBED, R], BF16, kind="ExternalInput")
    id_in = nc.dram_tensor("id_in", [128, 128], BF16, kind="ExternalInput")
    onescol_in = nc.dram_tensor("onescol_in", [1, 128], BF16, kind="ExternalInput")

    attn_o = nc.dram_tensor("attn_o", [NL, HEADS, QL, KL], BF16,
                            kind="ExternalOutput")
    out_o = nc.dram_tensor("out_o", [NL, QL, EMBED], F32, kind="ExternalOutput")

    with tile.TileContext(nc) as tc:
        with tc.tile_pool(name="persist", bufs=1) as persist, \
             tc.tile_pool(name="slab", bufs=1) as slab, \
             tc.tile_pool(name="proj", bufs=2) as proj, \
             tc.tile_pool(name="hdat", bufs=2) as hdat, \
             tc.tile_pool(name="setuppool", bufs=2) as setuppool, \
             tc.tile_pool(name="small", bufs=4) as small, \
             tc.tile_pool(name="pdpool", bufs=4) as pdpool, \
             tc.tile_pool(name="pospool", bufs=5) as pospool, \
             tc.tile_pool(name="outp", bufs=2) as outp, \
             tc.tile_pool(name="pp", bufs=2, space="PSUM") as pp, \
             tc.tile_pool(name="cc", bufs=2, space="PSUM") as cc, \
             tc.tile_pool(name="mm", bufs=2, space="PSUM") as mm, \
             tc.tile_pool(name="dscratch", bufs=5, space="DRAM") as dscratch:

            # ================= SETUP =================
            ident = persist.tile([128, 128], BF16)
            nc.sync.dma_start(out=ident[:], in_=id_in[:])
            onescol = persist.tile([1, 128], BF16)
            nc.sync.dma_start(out=onescol[:], in_=onescol_in[:])

            post = persist.tile([128, 4, R], BF16)  # [e within tile, etile, r]
            nc.sync.dma_start(
                out=post[:],
                in_=post_in[:].rearrange("(et e) r -> e et r", e=128))

            # per-partition bias vectors for head pairs: [128, HP]
            u_sb = persist.tile([128, HP], F32)
            vb_sb = persist.tile([128, HP], F32)
            for hp in range(HP):
                nc.sync.dma_start(out=u_sb[:, hp:hp + 1],
                                  in_=u_in[0, 128 * hp:128 * hp + 128])
                nc.sync.dma_start(out=vb_sb[:, hp:hp + 1],
                                  in_=vb_in[0, 128 * hp:128 * hp + 128])
            bo_b = persist.tile([1, EMBED], BF16)
            bo_f = setuppool.tile([1, EMBED], F32, tag="bof")
            nc.sync.dma_start(out=bo_f[:], in_=bo_in[:])
            nc.vector.tensor_copy(bo_b[:], bo_f[:])

            # f32 identity for PE transposes of f32 data
            idf = persist.tile([128, 128], F32)
            nc.vector.tensor_copy(idf[:], ident[:])

            # small weight transposes via PE transpose-mode
            bds = {}
            for name, w_dram in (("q", wq_in), ("k", wk_in), ("v", wv_in)):
                w_f = setuppool.tile([64, 64], F32, tag="w_f")
                nc.sync.dma_start(out=w_f[:], in_=w_dram[:])
                wtp = mm.tile([64, 64], F32, tag="mmt")
                nc.tensor.transpose(wtp[:], w_f[:], idf[0:64, 0:64])
                bd = persist.tile([128, 128], BF16, tag="bd" + name)
                nc.vector.memset(bd[:], 0.0)
                nc.vector.tensor_copy(bd[0:64, 0:64], wtp[:])
                nc.vector.tensor_copy(bd[64:128, 64:128], wtp[:])
                bds[name] = bd

            # Wr^T, Wo^T via PE transpose: layout [e(part), etile, e']
            wrt = persist.tile([128, 4, EMBED], BF16, tag="wrt")
            wot = persist.tile([128, 4, EMBED], BF16, tag="wot")
            for w_dram, dst in ((wr_in, wrt), (wo_in, wot)):
                for ot in range(4):  # e' tile (partition rows of source)
                    wrow_f = setuppool.tile([128, EMBED], F32, tag="wrow_f")
                    nc.sync.dma_start(out=wrow_f[:],
                                      in_=w_dram[128 * ot:128 * ot + 128, :])
                    wtp = mm.tile([128, 512], F32, tag="mmt")
                    for it in range(4):
                        nc.tensor.transpose(wtp[:, 128 * it:128 * it + 128],
                                            wrow_f[:, 128 * it:128 * it + 128],
                                            idf[:])
                    for it in range(4):
                        nc.scalar.activation(dst[:, it, 128 * ot:128 * ot + 128],
                                             wtp[:, 128 * it:128 * it + 128],
                                             AF.Identity)

            # rel table: rel_sb[e'g, e't, jcol] = rel[1023-jcol, e'] (bf16)
            rel_sb = persist.tile([128, 4, R], BF16, tag="rel")
            for et in range(4):
                for rh in range(2):
                    rpsum = mm.tile([128, 512], F32, tag="mmt")
                    for e in range(4):
                        nc.tensor.matmul(
                            rpsum[:],
                            wrt[:, e, 128 * et:128 * et + 128],
                            post[:, e, 512 * rh:512 * rh + 512],
                            start=(e == 0), stop=(e == 3))
                    nc.scalar.activation(
                        rel_sb[:, et, 512 * rh:512 * rh + 512], rpsum[:],
                        AF.Identity)

            # mask bias: (mask-1)*1e20 as bf16, [128, NL*4, 512]
            maskb = None
            if masked:
                maskb = persist.tile([128, NL * 4, 512], BF16, tag="maskb")
                for n in range(NL):
                    mrow = setuppool.tile([128, 4, 512], I32, tag="mrow")
                    nc.sync.dma_start(
                        out=mrow[:],
                        in_=m_in[n].rearrange("(qt p) k -> p qt k", p=128))
                    for qt in range(4):
                        nc.vector.tensor_scalar(
                            out=maskb[:, 4 * n + qt, :], in0=mrow[:, qt, :],
                            scalar1=-1.0, scalar2=1.0e20,
                            op0=ALU.add, op1=ALU.mult)

            # ============ per-batch state build ============
            def emit_slabs(n):
                """load x f32, SWDGE cast-write bf16 to DRAM, xbar-load
                transposed -> xt dict [e, etile, tok]"""
                out = {}
                for name, src in (("q", q_in), ("k", k_in), ("v", v_in)):
                    x_f = slab.tile([128, 4, EMBED], F32, tag="x_f")
                    nc.sync.dma_start(
                        out=x_f[:],
                        in_=src[n].rearrange("(tt t) e -> t tt e", t=128))
                    xscr = dscratch.tile([QL, EMBED], BF16, tag="xscr")
                    nc.gpsimd.dma_start(
                        out=xscr[:].rearrange("(tt p) e -> p tt e", p=128),
                        in_=x_f[:])
                    xt = slab.tile([128, 4, QL], BF16, tag="xt_" + name)
                    for et in range(4):
                        nc.scalar.dma_start_transpose(
                            out=xt[:, et, :],
                            in_=xscr[:, 128 * et:128 * et + 128])
                    out[name] = xt
                return out

            def emit_proj(n, xt):
                """projections for batch n -> (qu, qv, kt, v) tiles"""
                qu = proj.tile([128, HP, QL], BF16, tag="qu")
                qv = proj.tile([128, HP, QL], BF16, tag="qv")
                kt = proj.tile([128, HP, KL], BF16, tag="kt")
                vsb = proj.tile([128, 4, EMBED], BF16, tag="vsb")
                for hp in range(HP):
                    qp = mm.tile([128, QL], F32, tag="mmt")
                    nc.tensor.matmul(qp[:], bds["q"], xt["q"][:, hp, :],
                                     start=True, stop=True)
                    nc.vector.tensor_scalar(out=qu[:, hp, :], in0=qp[:],
                                            scalar1=u_sb[:, hp:hp + 1],
                                            scalar2=None, op0=ALU.add)
                    nc.vector.tensor_scalar(out=qv[:, hp, :], in0=qp[:],
                                            scalar1=vb_sb[:, hp:hp + 1],
                                            scalar2=None, op0=ALU.add)
                    kp = mm.tile([128, KL], F32, tag="mmt")
                    nc.tensor.matmul(kp[:], bds["k"], xt["k"][:, hp, :],
                                     start=True, stop=True)
                    nc.scalar.activation(kt[:, hp, :], kp[:], AF.Identity)
                    vp = mm.tile([128, 512], F32, tag="mmt")
                    for tt in range(4):
                        nc.tensor.matmul(vp[:, 128 * tt:128 * tt + 128],
                                         xt["v"][:, hp, 128 * tt:128 * tt + 128],
                                         bds["v"], start=True, stop=True)
                    for tt in range(4):
                        nc.scalar.activation(
                            vsb[:, tt, 128 * hp:128 * hp + 128],
                            vp[:, 128 * tt:128 * tt + 128], AF.Identity)
                return qu, qv, kt, vsb

            # ============ streamed attention ============
            def head_slice(t, h, qt=None):
                base = (h % 2) * 64
                if qt is None:
                    return t[base:base + 64, h // 2, :]
                return t[base:base + 64, h // 2, 128 * qt:128 * qt + 128]

            def emit_P(n, h, st):
                """position matmuls + drains + scratch write + diag read"""
                qv = st["proj"][1]
                rel_h = rel_sb[(h % 2) * 64:(h % 2) * 64 + 64, h // 2, :]
                pd4 = pdpool.tile([128, 4, 640], BF16, tag="pd4")
                for qt in range(4):
                    ws = 384 - 128 * qt
                    P = pp.tile([128, 640], F32, tag="pp")
                    nc.tensor.matmul(P[:, 0:512], head_slice(qv, h, qt),
                                     rel_h[:, ws:ws + 512], start=True, stop=True)
                    nc.tensor.matmul(P[:, 512:640], head_slice(qv, h, qt),
                                     rel_h[:, ws + 512:ws + 640],
                                     start=True, stop=True)
                    nc.vector.tensor_copy(pd4[:, qt, :], P[:])
                scr = dscratch.tile([4, 128, 640], BF16, tag="scr")
                # write [p, qt, j] -> [qt, p, j]
                nc.gpsimd.dma_start(
                    out=scr[:].rearrange("qt p j -> p qt j"), in_=pd4[:])
                # diagonal read: pos[p, qt, k] = scr[qt, p, 127 - p + k]
                pos4 = pospool.tile([128, 4, 512], BF16, tag="pos4")
                dsrc = bass.AP(scr[:].tensor, 127,
                               [[639, 128], [128 * 640, 4], [1, 512]])
                nc.gpsimd.dma_start(out=pos4[:], in_=dsrc)
                st["pos4"] = pos4

            def emit_C(n, h, st):
                qu, _, kt, _ = st["proj"]
                st["Z"] = small.tile([128, 4], F32, tag="Z", name="zt")
                st["expm"] = hdat.tile([128, 4, 512], BF16, tag="expm",
                                       name="expm")
                for qt in range(4):
                    C = cc.tile([128, 512], F32, tag="cc")
                    nc.tensor.matmul(C[:], head_slice(qu, h, qt),
                                     kt[(h % 2) * 64:(h % 2) * 64 + 64, h // 2, :],
                                     start=True, stop=False)
                    if masked:
                        nc.tensor.matmul(C[:], ident[:], maskb[:, 4 * n + qt, :],
                                         start=False, stop=False)
                    nc.tensor.matmul(C[:], ident[:], st["pos4"][:, qt, :],
                                     start=False, stop=True)
                    nc.scalar.activation(st["expm"][:, qt, :], C[:], AF.Exp,
                                         scale=0.125,
                                         accum_out=st["Z"][:, qt:qt + 1])

            def emit_F(n, h, st):
                rc = small.tile([128, 4], F32, tag="rc")
                nc.vector.reciprocal(rc[:], st["Z"][:])
                attn_n = hdat.tile([128, 4, 512], BF16, tag="attn_n")
                for qt in range(4):
                    norm_eng = nc.gpsimd if USE_POOL_NORM else nc.vector
                    norm_eng.tensor_scalar(
                        out=attn_n[:, qt, :], in0=st["expm"][:, qt, :],
                        scalar1=rc[:, qt:qt + 1], scalar2=None, op0=ALU.mult)
                # one batched output write [p, qt, k] -> attn[(qt p), k]
                nc.sync.dma_start(
                    out=attn_o[n, h].rearrange("(qt p) k -> p qt k", p=128),
                    in_=attn_n[:])
                # transposed reload from DRAM via xbar (ACT hwdge ring)
                attn_t = hdat.tile([128, 4, 512], BF16, tag="attn_t")
                for ktile in range(4):
                    eng = nc.scalar
                    eng.dma_start_transpose(
                        out=attn_t[:, ktile, :],
                        in_=attn_o[n, h][:, 128 * ktile:128 * ktile + 128])
                # ctx^T[d, q] = sum_k v[k, d] attn_t[k, q]
                vsb = st["proj"][3]
                ctxp = mm.tile([64, 512], F32, tag="mmt")
                for ktile in range(4):
                    nc.tensor.matmul(
                        ctxp[:],
                        vsb[:, ktile, 64 * h:64 * h + 64],
                        attn_t[:, ktile, :],
                        start=(ktile == 0), stop=(ktile == 3))
                nc.scalar.activation(
                    st["ctx"][(h % 2) * 64:(h % 2) * 64 + 64, h // 2, :],
                    ctxp[:], AF.Identity)

            def emit_E(n, st):
                osb = outp.tile([128, 4, 512], F32, tag="osb")
                for tt in range(4):
                    op = mm.tile([128, 512], F32, tag="mmt")
                    for et in range(4):
                        nc.tensor.matmul(
                            op[:], st["ctx"][:, et, 128 * tt:128 * tt + 128],
                            wot[:, et, :], start=(et == 0), stop=False)
                    nc.tensor.matmul(op[:], onescol[:], bo_b[:],
                                     start=False, stop=True)
                    nc.vector.tensor_copy(osb[:, tt, :], op[:])
                nc.sync.dma_start(
                    out=out_o[n].rearrange("(tt p) e -> p tt e", p=128),
                    in_=osb[:])

            # ---- main stream over heads with software pipelining ----
            SKEW = 4  # heads of lookahead between P and C phases
            heads = [(n, h) for n in range(NL) for h in range(HEADS)]
            state = {}
            fqueue = []
            cur_n = -1
            for i, (n, h) in enumerate(heads):
                if n != cur_n:
                    cur_n = n
                    xt = emit_slabs(n)
                    projt = emit_proj(n, xt)
                    ctx_sb = proj.tile([128, HP, QL], BF16, tag="ctx",
                                       name="ctx_sb")
                    nstate = {"proj": projt, "ctx": ctx_sb}
                state[(n, h)] = dict(nstate)
                emit_P(n, h, state[(n, h)])
                j = i - SKEW
                if j >= 0:
                    jn, jh = heads[j]
                    emit_C(jn, jh, state[(jn, jh)])
                    fqueue.append((jn, jh))
                    if len(fqueue) > 1:
                        fn, fh = fqueue.pop(0)
                        emit_F(fn, fh, state[(fn, fh)])
                        if fh == HEADS - 1:
                            emit_E(fn, state[(fn, fh)])
            for j in range(len(heads) - SKEW, len(heads)):
                jn, jh = heads[j]
                emit_C(jn, jh, state[(jn, jh)])
                fqueue.append((jn, jh))
            while fqueue:
                fn, fh = fqueue.pop(0)
                emit_F(fn, fh, state[(fn, fh)])
                if fh == HEADS - 1:
                    emit_E(fn, state[(fn, fh)])

    nc.compile()
    return nc


def _host_constants():
    invf = (10000.0 ** (-np.arange(0, EMBED, 2, dtype=np.float64) / EMBED))
    seq = np.arange(R, dtype=np.float64)[:, None] * invf[None, :]
    pos_emb = np.concatenate([np.sin(seq), np.cos(seq)], axis=-1)  # [R, E]
    post_rev = np.ascontiguousarray(pos_emb[::-1, :].T).astype(ml_dtypes.bfloat16)
    ident = np.eye(128, dtype=ml_dtypes.bfloat16)
    onescol = np.ones((1, 128), dtype=ml_dtypes.bfloat16)
    return post_rev, ident, onescol


def kernel(values, keys, query, mask, Wv, Wk, Wq, Wr, u_bias, v_bias, Wo, bo):
    values = np.ascontiguousarray(np.asarray(values, dtype=np.float32))
    keys = np.ascontiguousarray(np.asarray(keys, dtype=np.float32))
    query = np.ascontiguousarray(np.asarray(query, dtype=np.float32))
    mask = np.ascontiguousarray(np.asarray(mask, dtype=np.int32))
    Wv = np.ascontiguousarray(np.asarray(Wv, dtype=np.float32))
    Wk = np.ascontiguousarray(np.asarray(Wk, dtype=np.float32))
    Wq = np.ascontiguousarray(np.asarray(Wq, dtype=np.float32))
    Wr = np.ascontiguousarray(np.asarray(Wr, dtype=np.float32))
    Wo = np.ascontiguousarray(np.asarray(Wo, dtype=np.float32))
    u_flat = np.ascontiguousarray(np.asarray(u_bias, dtype=np.float32).reshape(1, EMBED))
    v_flat = np.ascontiguousarray(np.asarray(v_bias, dtype=np.float32).reshape(1, EMBED))
    bo_flat = np.ascontiguousarray(np.asarray(bo, dtype=np.float32).reshape(1, EMBED))

    post_rev, ident, onescol = _host_constants()

    masked = bool((mask == 0).any())
    nc = build_program(masked=masked)

    in_maps = []
    for c in range(NCORES):
        s = slice(NL * c, NL * (c + 1))
        in_maps.append({
            "q_in": query[s], "k_in": keys[s], "v_in": values[s],
            **({"m_in": mask[s]} if masked else {}),
            "wq_in": Wq, "wk_in": Wk, "wv_in": Wv, "wr_in": Wr, "wo_in": Wo,
            "u_in": u_flat, "vb_in": v_flat, "bo_in": bo_flat,
            "post_in": post_rev, "id_in": ident, "onescol_in": onescol,
        })

    res = run_bass_kernel_spmd(nc, in_maps, core_ids=list(range(NCORES)))
    global LAST_RESULTS
    LAST_RESULTS = res

    out = np.empty((NB, QL, EMBED), dtype=np.float32)
    attn = np.empty((NB, HEADS, QL, KL), dtype=np.float32)
    for c in range(NCORES):
        r = res.results[c]
        out[NL * c:NL * (c + 1)] = r["out_o"]
        attn[NL * c:NL * (c + 1)] = np.asarray(r["attn_o"], dtype=np.float32)
    return out, attn


# revision 12
# speedup vs baseline: 1.9233x; 1.0380x over previous
"""Transformer-XL style MultiHeadAttention on 8 TRN2 NeuronCores.

Data-parallel over batch N=16 -> 2 batches per core. Full computation on
device per core:
  - per-head projections q/k/v (shared 64x64 weights, block-diag over head
    pairs on the 128-wide PE contraction)
  - relative key table rel = pos_emb @ Wr.T computed on device (pos_emb is an
    input-independent sinusoid constant, host-precomputed, fed reversed and
    transposed so the Toeplitz shift becomes a forward-strided read)
  - energy = (q+u)k^T + (q+v)rel^T(shifted) + mask bias, softmax, attn@v,
    output projection with Wo/bo
  - the relative-position shift (energy[q,k] takes rel index q-k+512) is done
    by a DRAM roundtrip: P[128,4,640](bf16) -> one diagonal-strided 3D DMA
    read (row stride 639 elements) -> [128,4,512] aligned tiles ->
    identity-matmul accumulate into the content PSUM.

Matmuls run in bf16 (f32 PSUM accumulation); softmax in f32 on ACT with
fused row-sum (accum_out). Input/weight transposes run on the PE
(transpose mode); the attention transpose for attn@v reuses the attention
DRAM output via xbar DMA-transpose loads.
"""

import numpy as np
import ml_dtypes

import concourse.bass as bass
import concourse.mybir as mybir
import concourse.tile as tile
from concourse import bacc
from concourse.bass_utils import run_bass_kernel_spmd

F32 = mybir.dt.float32
BF16 = mybir.dt.bfloat16
I32 = mybir.dt.int32

EMBED = 512
HEADS = 8
HEAD_D = 64
NB = 16          # full batch
NL = 2           # batches per core
QL = 512
KL = 512
R = QL + KL      # 1024 relative positions
NCORES = 8
HP = HEADS // 2  # head pairs

LAST_RESULTS = None
USE_POOL_NORM = False
AF = mybir.ActivationFunctionType
ALU = mybir.AluOpType


def build_program(masked=True):
    nc = bacc.Bacc("TRN2", target_bir_lowering=False, debug=False)

    # ---- I/O ----
    q_in = nc.dram_tensor("q_in", [NL, QL, EMBED], F32, kind="ExternalInput")
    k_in = nc.dram_tensor("k_in", [NL, KL, EMBED], F32, kind="ExternalInput")
    v_in = nc.dram_tensor("v_in", [NL, KL, EMBED], F32, kind="ExternalInput")
    m_in = (nc.dram_tensor("m_in", [NL, QL, KL], I32, kind="ExternalInput")
            if masked else None)
    wq_in = nc.dram_tensor("wq_in", [HEAD_D, HEAD_D], F32, kind="ExternalInput")
    wk_in = nc.dram_tensor("wk_in", [HEAD_D, HEAD_D], F32, kind="ExternalInput")
    wv_in = nc.dram_tensor("wv_in", [HEAD_D, HEAD_D], F32, kind="ExternalInput")
    wr_in = nc.dram_tensor("wr_in", [EMBED, EMBED], F32, kind="ExternalInput")
    wo_in = nc.dram_tensor("wo_in", [EMBED, EMBED], F32, kind="ExternalInput")
    u_in = nc.dram_tensor("u_in", [1, EMBED], F32, kind="ExternalInput")
    vb_in = nc.dram_tensor("vb_in", [1, EMBED], F32, kind="ExternalInput")
    bo_in = nc.dram_tensor("bo_in", [1, EMBED], F32, kind="ExternalInput")
    post_in = nc.dram_tensor("post_in", [EMBED, R], BF16, kind="ExternalInput")
    id_in = nc.dram_tensor("id_in", [128, 128], BF16, kind="ExternalInput")
    onescol_in = nc.dram_tensor("onescol_in", [1, 128], BF16, kind="ExternalInput")

    attn_o = nc.dram_tensor("attn_o", [NL, HEADS, QL, KL], BF16,
                            kind="ExternalOutput")
    out_o = nc.dram_tensor("out_o", [NL, QL, EMBED], F32, kind="ExternalOutput")

    with tile.TileContext(nc) as tc:
        with tc.tile_pool(name="persist", bufs=1) as persist, \
             tc.tile_pool(name="slab", bufs=1) as slab, \
             tc.tile_pool(name="proj", bufs=2) as proj, \
             tc.tile_pool(name="hdat", bufs=4) as hdat, \
             tc.tile_pool(name="setuppool", bufs=2) as setuppool, \
             tc.tile_pool(name="small", bufs=4) as small, \
             tc.tile_pool(name="pdpool", bufs=3) as pdpool, \
             tc.tile_pool(name="pospool", bufs=4) as pospool, \
             tc.tile_pool(name="outp", bufs=2) as outp, \
             tc.tile_pool(name="pp", bufs=2, space="PSUM") as pp, \
             tc.tile_pool(name="cc", bufs=2, space="PSUM") as cc, \
             tc.tile_pool(name="mm", bufs=2, space="PSUM") as mm, \
             tc.tile_pool(name="dscratch", bufs=5, space="DRAM") as dscratch:

            # ================= SETUP =================
            ident = persist.tile([128, 128], BF16)
            nc.sync.dma_start(out=ident[:], in_=id_in[:])
            onescol = persist.tile([1, 128], BF16)
            nc.sync.dma_start(out=onescol[:], in_=onescol_in[:])

            post = persist.tile([128, 4, R], BF16)  # [e within tile, etile, r]
            nc.sync.dma_start(
                out=post[:],
                in_=post_in[:].rearrange("(et e) r -> e et r", e=128))

            # per-partition bias vectors for head pairs: [128, HP]
            u_sb = persist.tile([128, HP], F32)
            vb_sb = persist.tile([128, HP], F32)
            for hp in range(HP):
                nc.sync.dma_start(out=u_sb[:, hp:hp + 1],
                                  in_=u_in[0, 128 * hp:128 * hp + 128])
                nc.sync.dma_start(out=vb_sb[:, hp:hp + 1],
                                  in_=vb_in[0, 128 * hp:128 * hp + 128])
            bo_b = persist.tile([1, EMBED], BF16)
            bo_f = setuppool.tile([1, EMBED], F32, tag="bof")
            nc.sync.dma_start(out=bo_f[:], in_=bo_in[:])
            nc.vector.tensor_copy(bo_b[:], bo_f[:])

            # f32 identity for PE transposes of f32 data
            idf = persist.tile([128, 128], F32)
            nc.vector.tensor_copy(idf[:], ident[:])

            # small weight transposes via PE transpose-mode
            bds = {}
            for name, w_dram in (("q", wq_in), ("k", wk_in), ("v", wv_in)):
                w_f = setuppool.tile([64, 64], F32, tag="w_f")
                nc.sync.dma_start(out=w_f[:], in_=w_dram[:])
                wtp = mm.tile([64, 64], F32, tag="mmt")
                nc.tensor.transpose(wtp[:], w_f[:], idf[0:64, 0:64])
                bd = persist.tile([128, 128], BF16, tag="bd" + name)
                nc.vector.memset(bd[:], 0.0)
                nc.vector.tensor_copy(bd[0:64, 0:64], wtp[:])
                nc.vector.tensor_copy(bd[64:128, 64:128], wtp[:])
                bds[name] = bd

            # Wr^T, Wo^T via PE transpose: layout [e(part), etile, e']
            wrt = persist.tile([128, 4, EMBED], BF16, tag="wrt")
            wot = persist.tile([128, 4, EMBED], BF16, tag="wot")
            for w_dram, dst in ((wr_in, wrt), (wo_in, wot)):
                for ot in range(4):  # e' tile (partition rows of source)
                    wrow_f = setuppool.tile([128, EMBED], F32, tag="wrow_f")
                    nc.sync.dma_start(out=wrow_f[:],
                                      in_=w_dram[128 * ot:128 * ot + 128, :])
                    wtp = mm.tile([128, 512], F32, tag="mmt")
                    for it in range(4):
                        nc.tensor.transpose(wtp[:, 128 * it:128 * it + 128],
                                            wrow_f[:, 128 * it:128 * it + 128],
                                            idf[:])
                    for it in range(4):
                        nc.scalar.activation(dst[:, it, 128 * ot:128 * ot + 128],
                                             wtp[:, 128 * it:128 * it + 128],
                                             AF.Identity)

            # rel table: rel_sb[e'g, e't, jcol] = rel[1023-jcol, e'] (bf16)
            rel_sb = persist.tile([128, 4, R], BF16, tag="rel")
            for et in range(4):
                for rh in range(2):
                    rpsum = mm.tile([128, 512], F32, tag="mmt")
                    for e in range(4):
                        nc.tensor.matmul(
                            rpsum[:],
                            wrt[:, e, 128 * et:128 * et + 128],
                            post[:, e, 512 * rh:512 * rh + 512],
                            start=(e == 0), stop=(e == 3))
                    nc.scalar.activation(
                        rel_sb[:, et, 512 * rh:512 * rh + 512], rpsum[:],
                        AF.Identity)

            # mask bias: (mask-1)*1e20 as bf16, [128, NL*4, 512]
            maskb = None
            if masked:
                maskb = persist.tile([128, NL * 4, 512], BF16, tag="maskb")
                for n in range(NL):
                    mrow = setuppool.tile([128, 4, 512], I32, tag="mrow")
                    nc.sync.dma_start(
                        out=mrow[:],
                        in_=m_in[n].rearrange("(qt p) k -> p qt k", p=128))
                    for qt in range(4):
                        nc.vector.tensor_scalar(
                            out=maskb[:, 4 * n + qt, :], in0=mrow[:, qt, :],
                            scalar1=-1.0, scalar2=1.0e20,
                            op0=ALU.add, op1=ALU.mult)

            # ============ per-batch state build ============
            def emit_slabs(n):
                """load x f32, SWDGE cast-write bf16 to DRAM, xbar-load
                transposed -> xt dict [e, etile, tok]"""
                out = {}
                for name, src in (("q", q_in), ("k", k_in), ("v", v_in)):
                    x_f = slab.tile([128, 4, EMBED], F32, tag="x_f")
                    nc.gpsimd.dma_start(
                        out=x_f[:],
                        in_=src[n].rearrange("(tt t) e -> t tt e", t=128))
                    xscr = dscratch.tile([QL, EMBED], BF16, tag="xscr")
                    nc.gpsimd.dma_start(
                        out=xscr[:].rearrange("(tt p) e -> p tt e", p=128),
                        in_=x_f[:])
                    xt = slab.tile([128, 4, QL], BF16, tag="xt_" + name)
                    for et in range(4):
                        eng = nc.scalar if et % 2 == 0 else nc.sync
                        eng.dma_start_transpose(
                            out=xt[:, et, :],
                            in_=xscr[:, 128 * et:128 * et + 128])
                    out[name] = xt
                return out

            def emit_proj(n, xt):
                """projections for batch n -> (qu, qv, kt, v) tiles"""
                qu = proj.tile([128, HP, QL], BF16, tag="qu")
                qv = proj.tile([128, HP, QL], BF16, tag="qv")
                kt = proj.tile([128, HP, KL], BF16, tag="kt")
                vsb = proj.tile([128, 4, EMBED], BF16, tag="vsb")
                for hp in range(HP):
                    qp = mm.tile([128, QL], F32, tag="mmt")
                    nc.tensor.matmul(qp[:], bds["q"], xt["q"][:, hp, :],
                                     start=True, stop=True)
                    nc.vector.tensor_scalar(out=qu[:, hp, :], in0=qp[:],
                                            scalar1=u_sb[:, hp:hp + 1],
                                            scalar2=None, op0=ALU.add)
                    nc.vector.tensor_scalar(out=qv[:, hp, :], in0=qp[:],
                                            scalar1=vb_sb[:, hp:hp + 1],
                                            scalar2=None, op0=ALU.add)
                    kp = mm.tile([128, KL], F32, tag="mmt")
                    nc.tensor.matmul(kp[:], bds["k"], xt["k"][:, hp, :],
                                     start=True, stop=True)
                    nc.scalar.activation(kt[:, hp, :], kp[:], AF.Identity)
                    vp = mm.tile([128, 512], F32, tag="mmt")
                    for tt in range(4):
                        nc.tensor.matmul(vp[:, 128 * tt:128 * tt + 128],
                                         xt["v"][:, hp, 128 * tt:128 * tt + 128],
                                         bds["v"], start=True, stop=True)
                    for tt in range(4):
                        nc.vector.tensor_copy(
                            vsb[:, tt, 128 * hp:128 * hp + 128],
                            vp[:, 128 * tt:128 * tt + 128])
                return qu, qv, kt, vsb

            # ============ streamed attention ============
            def head_slice(t, h, qt=None):
                base = (h % 2) * 64
                if qt is None:
                    return t[base:base + 64, h // 2, :]
                return t[base:base + 64, h // 2, 128 * qt:128 * qt + 128]

            def emit_P(n, h, st):
                """position matmuls + drains + scratch write + diag read"""
                qv = st["proj"][1]
                rel_h = rel_sb[(h % 2) * 64:(h % 2) * 64 + 64, h // 2, :]
                pd4 = pdpool.tile([128, 4, 640], BF16, tag="pd4")
                for qt in range(4):
                    ws = 384 - 128 * qt
                    P = pp.tile([128, 640], F32, tag="pp")
                    nc.tensor.matmul(P[:, 0:512], head_slice(qv, h, qt),
                                     rel_h[:, ws:ws + 512], start=True, stop=True)
                    nc.tensor.matmul(P[:, 512:640], head_slice(qv, h, qt),
                                     rel_h[:, ws + 512:ws + 640],
                                     start=True, stop=True)
                    nc.vector.tensor_copy(pd4[:, qt, :], P[:])
                scr = dscratch.tile([4, 128, 640], BF16, tag="scr")
                # write [p, qt, j] -> [qt, p, j]
                nc.gpsimd.dma_start(
                    out=scr[:].rearrange("qt p j -> p qt j"), in_=pd4[:])
                # diagonal read: pos[p, qt, k] = scr[qt, p, 127 - p + k]
                pos4 = pospool.tile([128, 4, 512], BF16, tag="pos4")
                dsrc = bass.AP(scr[:].tensor, 127,
                               [[639, 128], [128 * 640, 4], [1, 512]])
                nc.gpsimd.dma_start(out=pos4[:], in_=dsrc)
                st["pos4"] = pos4

            def emit_C(n, h, st):
                qu, _, kt, _ = st["proj"]
                st["Z"] = small.tile([128, 4], F32, tag="Z", name="zt")
                st["expm"] = hdat.tile([128, 4, 512], BF16, tag="expm",
                                       name="expm")
                for qt in range(4):
                    C = cc.tile([128, 512], F32, tag="cc")
                    nc.tensor.matmul(C[:], head_slice(qu, h, qt),
                                     kt[(h % 2) * 64:(h % 2) * 64 + 64, h // 2, :],
                                     start=True, stop=False)
                    if masked:
                        nc.tensor.matmul(C[:], ident[:], maskb[:, 4 * n + qt, :],
                                         start=False, stop=False)
                    nc.tensor.matmul(C[:], ident[:], st["pos4"][:, qt, :],
                                     start=False, stop=True)
                    nc.scalar.activation(st["expm"][:, qt, :], C[:], AF.Exp,
                                         scale=0.125,
                                         accum_out=st["Z"][:, qt:qt + 1])

            def emit_F(n, h, st):
                rc = small.tile([128, 4], F32, tag="rc")
                nc.vector.reciprocal(rc[:], st["Z"][:])
                attn_n = st["expm"]
                for qt in range(4):
                    nc.vector.tensor_scalar(
                        out=attn_n[:, qt, :], in0=st["expm"][:, qt, :],
                        scalar1=rc[:, qt:qt + 1], scalar2=None, op0=ALU.mult)
                # one batched output write [p, qt, k] -> attn[(qt p), k]
                nc.gpsimd.dma_start(
                    out=attn_o[n, h].rearrange("(qt p) k -> p qt k", p=128),
                    in_=attn_n[:])
                # transposed reload from DRAM via xbar (ACT hwdge ring)
                attn_t = hdat.tile([128, 4, 512], BF16, tag="attn_t")
                for ktile in range(4):
                    eng = nc.scalar
                    eng.dma_start_transpose(
                        out=attn_t[:, ktile, :],
                        in_=attn_o[n, h][:, 128 * ktile:128 * ktile + 128])
                # ctx^T[d, q] = sum_k v[k, d] attn_t[k, q]
                vsb = st["proj"][3]
                ctxp = mm.tile([64, 512], F32, tag="mmt")
                for ktile in range(4):
                    nc.tensor.matmul(
                        ctxp[:],
                        vsb[:, ktile, 64 * h:64 * h + 64],
                        attn_t[:, ktile, :],
                        start=(ktile == 0), stop=(ktile == 3))
                nc.scalar.activation(
                    st["ctx"][(h % 2) * 64:(h % 2) * 64 + 64, h // 2, :],
                    ctxp[:], AF.Identity)

            def emit_E(n, st):
                osb = outp.tile([128, 4, 512], F32, tag="osb")
                for tt in range(4):
                    op = mm.tile([128, 512], F32, tag="mmt")
                    for et in range(4):
                        nc.tensor.matmul(
                            op[:], st["ctx"][:, et, 128 * tt:128 * tt + 128],
                            wot[:, et, :], start=(et == 0), stop=False)
                    nc.tensor.matmul(op[:], onescol[:], bo_b[:],
                                     start=False, stop=True)
                    nc.vector.tensor_copy(osb[:, tt, :], op[:])
                nc.gpsimd.dma_start(
                    out=out_o[n].rearrange("(tt p) e -> p tt e", p=128),
                    in_=osb[:])

            # ---- main stream over heads with software pipelining ----
            SKEW = 4  # heads of lookahead between P and C phases
            heads = [(n, h) for n in range(NL) for h in range(HEADS)]
            state = {}
            fqueue = []
            cur_n = -1
            for i, (n, h) in enumerate(heads):
                if n != cur_n:
                    cur_n = n
                    xt = emit_slabs(n)
                    projt = emit_proj(n, xt)
                    ctx_sb = proj.tile([128, HP, QL], BF16, tag="ctx",
                                       name="ctx_sb")
                    nstate = {"proj": projt, "ctx": ctx_sb}
                state[(n, h)] = dict(nstate)
                emit_P(n, h, state[(n, h)])
                j = i - SKEW
                if j >= 0:
                    jn, jh = heads[j]
                    emit_C(jn, jh, state[(jn, jh)])
                    fqueue.append((jn, jh))
                    if len(fqueue) > 1:
                        fn, fh = fqueue.pop(0)
                        emit_F(fn, fh, state[(fn, fh)])
                        if fh == HEADS - 1:
                            emit_E(fn, state[(fn, fh)])
            for j in range(len(heads) - SKEW, len(heads)):
                jn, jh = heads[j]
                emit_C(jn, jh, state[(jn, jh)])
                fqueue.append((jn, jh))
            while fqueue:
                fn, fh = fqueue.pop(0)
                emit_F(fn, fh, state[(fn, fh)])
                if fh == HEADS - 1:
                    emit_E(fn, state[(fn, fh)])

    nc.compile()
    return nc


def _host_constants():
    invf = (10000.0 ** (-np.arange(0, EMBED, 2, dtype=np.float64) / EMBED))
    seq = np.arange(R, dtype=np.float64)[:, None] * invf[None, :]
    pos_emb = np.concatenate([np.sin(seq), np.cos(seq)], axis=-1)  # [R, E]
    post_rev = np.ascontiguousarray(pos_emb[::-1, :].T).astype(ml_dtypes.bfloat16)
    ident = np.eye(128, dtype=ml_dtypes.bfloat16)
    onescol = np.ones((1, 128), dtype=ml_dtypes.bfloat16)
    return post_rev, ident, onescol


def kernel(values, keys, query, mask, Wv, Wk, Wq, Wr, u_bias, v_bias, Wo, bo):
    values = np.ascontiguousarray(np.asarray(values, dtype=np.float32))
    keys = np.ascontiguousarray(np.asarray(keys, dtype=np.float32))
    query = np.ascontiguousarray(np.asarray(query, dtype=np.float32))
    mask = np.ascontiguousarray(np.asarray(mask, dtype=np.int32))
    Wv = np.ascontiguousarray(np.asarray(Wv, dtype=np.float32))
    Wk = np.ascontiguousarray(np.asarray(Wk, dtype=np.float32))
    Wq = np.ascontiguousarray(np.asarray(Wq, dtype=np.float32))
    Wr = np.ascontiguousarray(np.asarray(Wr, dtype=np.float32))
    Wo = np.ascontiguousarray(np.asarray(Wo, dtype=np.float32))
    u_flat = np.ascontiguousarray(np.asarray(u_bias, dtype=np.float32).reshape(1, EMBED))
    v_flat = np.ascontiguousarray(np.asarray(v_bias, dtype=np.float32).reshape(1, EMBED))
    bo_flat = np.ascontiguousarray(np.asarray(bo, dtype=np.float32).reshape(1, EMBED))

    post_rev, ident, onescol = _host_constants()

    masked = bool((mask == 0).any())
    nc = build_program(masked=masked)

    in_maps = []
    for c in range(NCORES):
        s = slice(NL * c, NL * (c + 1))
        in_maps.append({
            "q_in": query[s], "k_in": keys[s], "v_in": values[s],
            **({"m_in": mask[s]} if masked else {}),
            "wq_in": Wq, "wk_in": Wk, "wv_in": Wv, "wr_in": Wr, "wo_in": Wo,
            "u_in": u_flat, "vb_in": v_flat, "bo_in": bo_flat,
            "post_in": post_rev, "id_in": ident, "onescol_in": onescol,
        })

    res = run_bass_kernel_spmd(nc, in_maps, core_ids=list(range(NCORES)))
    global LAST_RESULTS
    LAST_RESULTS = res

    out = np.empty((NB, QL, EMBED), dtype=np.float32)
    attn = np.empty((NB, HEADS, QL, KL), dtype=np.float32)
    for c in range(NCORES):
        r = res.results[c]
        out[NL * c:NL * (c + 1)] = r["out_o"]
        attn[NL * c:NL * (c + 1)] = np.asarray(r["attn_o"], dtype=np.float32)
    return out, attn


# revision 13
# speedup vs baseline: 2.2247x; 1.1567x over previous
"""Transformer-XL style MultiHeadAttention on 8 TRN2 NeuronCores.

Data-parallel over batch N=16 -> 2 batches per core. Full computation on
device per core:
  - per-head projections q/k/v (shared 64x64 weights, block-diag over head
    pairs on the 128-wide PE contraction)
  - relative key table rel = pos_emb @ Wr.T computed on device (pos_emb is an
    input-independent sinusoid constant, host-precomputed, fed reversed and
    transposed so the Toeplitz shift becomes a forward-strided read)
  - energy = (q+u)k^T + (q+v)rel^T(shifted) + mask bias, softmax, attn@v,
    output projection with Wo/bo
  - the relative-position shift (energy[q,k] takes rel index q-k+512) is done
    by a DRAM roundtrip: P[128,4,640](bf16) -> one diagonal-strided 3D DMA
    read (row stride 639 elements) -> [128,4,512] aligned tiles ->
    identity-matmul accumulate into the content PSUM.

Matmuls run in bf16 (f32 PSUM accumulation); softmax in f32 on ACT with
fused row-sum (accum_out). Input/weight transposes run on the PE
(transpose mode); the attention transpose for attn@v reuses the attention
DRAM output via xbar DMA-transpose loads.
"""

import numpy as np
import ml_dtypes

import concourse.bass as bass
import concourse.mybir as mybir
import concourse.tile as tile
from concourse import bacc
from concourse.bass_utils import run_bass_kernel_spmd

F32 = mybir.dt.float32
BF16 = mybir.dt.bfloat16
I32 = mybir.dt.int32

EMBED = 512
HEADS = 8
HEAD_D = 64
NB = 16          # full batch
NL = 2           # batches per core
QL = 512
KL = 512
R = QL + KL      # 1024 relative positions
NCORES = 8
HP = HEADS // 2  # head pairs

LAST_RESULTS = None
USE_POOL_NORM = False
AF = mybir.ActivationFunctionType
ALU = mybir.AluOpType


def build_program(masked=True):
    nc = bacc.Bacc("TRN2", target_bir_lowering=False, debug=False)

    # ---- I/O ----
    q_in = nc.dram_tensor("q_in", [NL, QL, EMBED], F32, kind="ExternalInput")
    k_in = nc.dram_tensor("k_in", [NL, KL, EMBED], F32, kind="ExternalInput")
    v_in = nc.dram_tensor("v_in", [NL, KL, EMBED], F32, kind="ExternalInput")
    m_in = (nc.dram_tensor("m_in", [NL, QL, KL], I32, kind="ExternalInput")
            if masked else None)
    wq_in = nc.dram_tensor("wq_in", [HEAD_D, HEAD_D], F32, kind="ExternalInput")
    wk_in = nc.dram_tensor("wk_in", [HEAD_D, HEAD_D], F32, kind="ExternalInput")
    wv_in = nc.dram_tensor("wv_in", [HEAD_D, HEAD_D], F32, kind="ExternalInput")
    wr_in = nc.dram_tensor("wr_in", [EMBED, EMBED], F32, kind="ExternalInput")
    wo_in = nc.dram_tensor("wo_in", [EMBED, EMBED], F32, kind="ExternalInput")
    u_in = nc.dram_tensor("u_in", [1, EMBED], F32, kind="ExternalInput")
    vb_in = nc.dram_tensor("vb_in", [1, EMBED], F32, kind="ExternalInput")
    bo_in = nc.dram_tensor("bo_in", [1, EMBED], F32, kind="ExternalInput")
    post_in = nc.dram_tensor("post_in", [EMBED, R], BF16, kind="ExternalInput")
    id_in = nc.dram_tensor("id_in", [128, 128], BF16, kind="ExternalInput")
    onescol_in = nc.dram_tensor("onescol_in", [1, 128], BF16, kind="ExternalInput")

    attn_o = nc.dram_tensor("attn_o", [NL, HEADS, QL, KL], BF16,
                            kind="ExternalOutput")
    out_o = nc.dram_tensor("out_o", [NL, QL, EMBED], F32, kind="ExternalOutput")

    with tile.TileContext(nc) as tc:
        with tc.tile_pool(name="persist", bufs=1) as persist, \
             tc.tile_pool(name="slab", bufs=1) as slab, \
             tc.tile_pool(name="proj", bufs=2) as proj, \
             tc.tile_pool(name="hdat", bufs=4) as hdat, \
             tc.tile_pool(name="setuppool", bufs=2) as setuppool, \
             tc.tile_pool(name="small", bufs=4) as small, \
             tc.tile_pool(name="pdpool", bufs=3) as pdpool, \
             tc.tile_pool(name="pospool", bufs=4) as pospool, \
             tc.tile_pool(name="outp", bufs=2) as outp, \
             tc.tile_pool(name="pp", bufs=2, space="PSUM") as pp, \
             tc.tile_pool(name="cc", bufs=2, space="PSUM") as cc, \
             tc.tile_pool(name="mm", bufs=2, space="PSUM") as mm, \
             tc.tile_pool(name="dscratch", bufs=5, space="DRAM") as dscratch:

            # ================= SETUP =================
            ident = persist.tile([128, 128], BF16)
            nc.sync.dma_start(out=ident[:], in_=id_in[:])
            onescol = persist.tile([1, 128], BF16)
            nc.sync.dma_start(out=onescol[:], in_=onescol_in[:])

            post = persist.tile([128, 4, R], BF16)  # [e within tile, etile, r]
            nc.sync.dma_start(
                out=post[:],
                in_=post_in[:].rearrange("(et e) r -> e et r", e=128))

            # per-partition bias vectors for head pairs: [128, HP]
            u_sb = persist.tile([128, HP], F32)
            vb_sb = persist.tile([128, HP], F32)
            for hp in range(HP):
                nc.sync.dma_start(out=u_sb[:, hp:hp + 1],
                                  in_=u_in[0, 128 * hp:128 * hp + 128])
                nc.sync.dma_start(out=vb_sb[:, hp:hp + 1],
                                  in_=vb_in[0, 128 * hp:128 * hp + 128])
            bo_b = persist.tile([1, EMBED], BF16)
            bo_f = setuppool.tile([1, EMBED], F32, tag="bof")
            nc.sync.dma_start(out=bo_f[:], in_=bo_in[:])
            nc.vector.tensor_copy(bo_b[:], bo_f[:])

            # f32 identity for PE transposes of f32 data
            idf = persist.tile([128, 128], F32)
            nc.vector.tensor_copy(idf[:], ident[:])

            # small weight transposes via PE transpose-mode
            bds = {}
            for name, w_dram in (("q", wq_in), ("k", wk_in), ("v", wv_in)):
                w_f = setuppool.tile([64, 64], F32, tag="w_f")
                nc.sync.dma_start(out=w_f[:], in_=w_dram[:])
                wtp = mm.tile([64, 64], F32, tag="mmt")
                nc.tensor.transpose(wtp[:], w_f[:], idf[0:64, 0:64])
                bd = persist.tile([128, 128], BF16, tag="bd" + name)
                nc.vector.memset(bd[:], 0.0)
                nc.vector.tensor_copy(bd[0:64, 0:64], wtp[:])
                nc.vector.tensor_copy(bd[64:128, 64:128], wtp[:])
                bds[name] = bd

            # Wr^T, Wo^T via PE transpose: layout [e(part), etile, e']
            wrt = persist.tile([128, 4, EMBED], BF16, tag="wrt")
            wot = persist.tile([128, 4, EMBED], BF16, tag="wot")
            for w_dram, dst in ((wr_in, wrt), (wo_in, wot)):
                for ot in range(4):  # e' tile (partition rows of source)
                    wrow_f = setuppool.tile([128, EMBED], F32, tag="wrow_f")
                    nc.sync.dma_start(out=wrow_f[:],
                                      in_=w_dram[128 * ot:128 * ot + 128, :])
                    wtp = mm.tile([128, 512], F32, tag="mmt")
                    for it in range(4):
                        nc.tensor.transpose(wtp[:, 128 * it:128 * it + 128],
                                            wrow_f[:, 128 * it:128 * it + 128],
                                            idf[:])
                    for it in range(4):
                        nc.scalar.activation(dst[:, it, 128 * ot:128 * ot + 128],
                                             wtp[:, 128 * it:128 * it + 128],
                                             AF.Identity)

            # rel table: rel_sb[e'g, e't, jcol] = rel[1023-jcol, e'] (bf16)
            rel_sb = persist.tile([128, 4, R], BF16, tag="rel")
            for et in range(4):
                for rh in range(2):
                    rpsum = mm.tile([128, 512], F32, tag="mmt")
                    for e in range(4):
                        nc.tensor.matmul(
                            rpsum[:],
                            wrt[:, e, 128 * et:128 * et + 128],
                            post[:, e, 512 * rh:512 * rh + 512],
                            start=(e == 0), stop=(e == 3))
                    nc.scalar.activation(
                        rel_sb[:, et, 512 * rh:512 * rh + 512], rpsum[:],
                        AF.Identity)

            # mask bias: (mask-1)*1e20 as bf16, [128, NL*4, 512]
            maskb = None
            if masked:
                maskb = persist.tile([128, NL * 4, 512], BF16, tag="maskb")
                for n in range(NL):
                    mrow = setuppool.tile([128, 4, 512], I32, tag="mrow")
                    nc.sync.dma_start(
                        out=mrow[:],
                        in_=m_in[n].rearrange("(qt p) k -> p qt k", p=128))
                    for qt in range(4):
                        nc.vector.tensor_scalar(
                            out=maskb[:, 4 * n + qt, :], in0=mrow[:, qt, :],
                            scalar1=-1.0, scalar2=1.0e20,
                            op0=ALU.add, op1=ALU.mult)

            # ============ per-batch state build ============
            def emit_slabs(n):
                """load x f32, SWDGE cast-write bf16 to DRAM, xbar-load
                transposed -> xt dict [e, etile, tok]"""
                out = {}
                for name, src in (("q", q_in), ("k", k_in), ("v", v_in)):
                    x_f = slab.tile([128, 4, EMBED], F32, tag="x_f")
                    nc.gpsimd.dma_start(
                        out=x_f[:],
                        in_=src[n].rearrange("(tt t) e -> t tt e", t=128))
                    xscr = dscratch.tile([QL, EMBED], BF16, tag="xscr")
                    nc.gpsimd.dma_start(
                        out=xscr[:].rearrange("(tt p) e -> p tt e", p=128),
                        in_=x_f[:])
                    xt = slab.tile([128, 4, QL], BF16, tag="xt_" + name)
                    nc.scalar.dma_start_transpose(out=xt[:], in_=xscr[:])
                    out[name] = xt
                return out

            def emit_proj(n, xt):
                """projections for batch n -> (qu, qv, kt, v) tiles"""
                qu = proj.tile([128, HP, QL], BF16, tag="qu")
                qv = proj.tile([128, HP, QL], BF16, tag="qv")
                kt = proj.tile([128, HP, KL], BF16, tag="kt")
                vsb = proj.tile([128, 4, EMBED], BF16, tag="vsb")
                for hp in range(HP):
                    qp = mm.tile([128, QL], F32, tag="mmt")
                    nc.tensor.matmul(qp[:], bds["q"], xt["q"][:, hp, :],
                                     start=True, stop=True)
                    nc.vector.tensor_scalar(out=qu[:, hp, :], in0=qp[:],
                                            scalar1=u_sb[:, hp:hp + 1],
                                            scalar2=None, op0=ALU.add)
                    nc.vector.tensor_scalar(out=qv[:, hp, :], in0=qp[:],
                                            scalar1=vb_sb[:, hp:hp + 1],
                                            scalar2=None, op0=ALU.add)
                    kp = mm.tile([128, KL], F32, tag="mmt")
                    nc.tensor.matmul(kp[:], bds["k"], xt["k"][:, hp, :],
                                     start=True, stop=True)
                    nc.scalar.activation(kt[:, hp, :], kp[:], AF.Identity)
                    vp = mm.tile([128, 512], F32, tag="mmt")
                    for tt in range(4):
                        nc.tensor.matmul(vp[:, 128 * tt:128 * tt + 128],
                                         xt["v"][:, hp, 128 * tt:128 * tt + 128],
                                         bds["v"], start=True, stop=True)
                    for tt in range(4):
                        nc.vector.tensor_copy(
                            vsb[:, tt, 128 * hp:128 * hp + 128],
                            vp[:, 128 * tt:128 * tt + 128])
                return qu, qv, kt, vsb

            # ============ streamed attention ============
            def head_slice(t, h, qt=None):
                base = (h % 2) * 64
                if qt is None:
                    return t[base:base + 64, h // 2, :]
                return t[base:base + 64, h // 2, 128 * qt:128 * qt + 128]

            def emit_P(n, h, st):
                """position matmuls + drains + scratch write + diag read"""
                qv = st["proj"][1]
                rel_h = rel_sb[(h % 2) * 64:(h % 2) * 64 + 64, h // 2, :]
                pd4 = pdpool.tile([128, 4, 640], BF16, tag="pd4")
                for qt in range(4):
                    ws = 384 - 128 * qt
                    P = pp.tile([128, 640], F32, tag="pp")
                    nc.tensor.matmul(P[:, 0:512], head_slice(qv, h, qt),
                                     rel_h[:, ws:ws + 512], start=True, stop=True)
                    nc.tensor.matmul(P[:, 512:640], head_slice(qv, h, qt),
                                     rel_h[:, ws + 512:ws + 640],
                                     start=True, stop=True)
                    nc.vector.tensor_copy(pd4[:, qt, :], P[:])
                scr = dscratch.tile([4, 128, 640], BF16, tag="scr")
                # write [p, qt, j] -> [qt, p, j]
                nc.gpsimd.dma_start(
                    out=scr[:].rearrange("qt p j -> p qt j"), in_=pd4[:])
                # diagonal read: pos[p, qt, k] = scr[qt, p, 127 - p + k]
                pos4 = pospool.tile([128, 4, 512], BF16, tag="pos4")
                dsrc = bass.AP(scr[:].tensor, 127,
                               [[639, 128], [128 * 640, 4], [1, 512]])
                nc.gpsimd.dma_start(out=pos4[:], in_=dsrc)
                st["pos4"] = pos4

            def emit_C(n, h, st):
                qu, _, kt, _ = st["proj"]
                st["Z"] = small.tile([128, 4], F32, tag="Z", name="zt")
                st["expm"] = hdat.tile([128, 4, 512], BF16, tag="expm",
                                       name="expm")
                for qt in range(4):
                    C = cc.tile([128, 512], F32, tag="cc")
                    nc.tensor.matmul(C[:], head_slice(qu, h, qt),
                                     kt[(h % 2) * 64:(h % 2) * 64 + 64, h // 2, :],
                                     start=True, stop=False)
                    if masked:
                        nc.tensor.matmul(C[:], ident[:], maskb[:, 4 * n + qt, :],
                                         start=False, stop=False)
                    nc.tensor.matmul(C[:], ident[:], st["pos4"][:, qt, :],
                                     start=False, stop=True)
                    nc.scalar.activation(st["expm"][:, qt, :], C[:], AF.Exp,
                                         scale=0.125,
                                         accum_out=st["Z"][:, qt:qt + 1])

            def emit_F(n, h, st):
                rc = small.tile([128, 4], F32, tag="rc")
                nc.vector.reciprocal(rc[:], st["Z"][:])
                attn_n = st["expm"]
                for qt in range(4):
                    nc.vector.tensor_scalar(
                        out=attn_n[:, qt, :], in0=st["expm"][:, qt, :],
                        scalar1=rc[:, qt:qt + 1], scalar2=None, op0=ALU.mult)
                # one batched output write [p, qt, k] -> attn[(qt p), k]
                nc.gpsimd.dma_start(
                    out=attn_o[n, h].rearrange("(qt p) k -> p qt k", p=128),
                    in_=attn_n[:])
                # transposed reload from DRAM via xbar (ACT hwdge ring)
                attn_t = hdat.tile([128, 4, 512], BF16, tag="attn_t")
                nc.scalar.dma_start_transpose(out=attn_t[:], in_=attn_o[n, h][:])
                # ctx^T[d, q] = sum_k v[k, d] attn_t[k, q]
                vsb = st["proj"][3]
                ctxp = mm.tile([64, 512], F32, tag="mmt")
                for ktile in range(4):
                    nc.tensor.matmul(
                        ctxp[:],
                        vsb[:, ktile, 64 * h:64 * h + 64],
                        attn_t[:, ktile, :],
                        start=(ktile == 0), stop=(ktile == 3))
                nc.scalar.activation(
                    st["ctx"][(h % 2) * 64:(h % 2) * 64 + 64, h // 2, :],
                    ctxp[:], AF.Identity)

            def emit_E(n, st):
                osb = outp.tile([128, 4, 512], F32, tag="osb")
                for tt in range(4):
                    op = mm.tile([128, 512], F32, tag="mmt")
                    for et in range(4):
                        nc.tensor.matmul(
                            op[:], st["ctx"][:, et, 128 * tt:128 * tt + 128],
                            wot[:, et, :], start=(et == 0), stop=False)
                    nc.tensor.matmul(op[:], onescol[:], bo_b[:],
                                     start=False, stop=True)
                    nc.vector.tensor_copy(osb[:, tt, :], op[:])
                nc.gpsimd.dma_start(
                    out=out_o[n].rearrange("(tt p) e -> p tt e", p=128),
                    in_=osb[:])

            # ---- main stream over heads with software pipelining ----
            SKEW = 4  # heads of lookahead between P and C phases
            heads = [(n, h) for n in range(NL) for h in range(HEADS)]
            state = {}
            fqueue = []
            cur_n = -1
            for i, (n, h) in enumerate(heads):
                if n != cur_n:
                    cur_n = n
                    xt = emit_slabs(n)
                    projt = emit_proj(n, xt)
                    ctx_sb = proj.tile([128, HP, QL], BF16, tag="ctx",
                                       name="ctx_sb")
                    nstate = {"proj": projt, "ctx": ctx_sb}
                state[(n, h)] = dict(nstate)
                emit_P(n, h, state[(n, h)])
                j = i - SKEW
                if j >= 0:
                    jn, jh = heads[j]
                    emit_C(jn, jh, state[(jn, jh)])
                    fqueue.append((jn, jh))
                    if len(fqueue) > 1:
                        fn, fh = fqueue.pop(0)
                        emit_F(fn, fh, state[(fn, fh)])
                        if fh == HEADS - 1:
                            emit_E(fn, state[(fn, fh)])
            for j in range(len(heads) - SKEW, len(heads)):
                jn, jh = heads[j]
                emit_C(jn, jh, state[(jn, jh)])
                fqueue.append((jn, jh))
            while fqueue:
                fn, fh = fqueue.pop(0)
                emit_F(fn, fh, state[(fn, fh)])
                if fh == HEADS - 1:
                    emit_E(fn, state[(fn, fh)])

    nc.compile()
    return nc


def _host_constants():
    invf = (10000.0 ** (-np.arange(0, EMBED, 2, dtype=np.float64) / EMBED))
    seq = np.arange(R, dtype=np.float64)[:, None] * invf[None, :]
    pos_emb = np.concatenate([np.sin(seq), np.cos(seq)], axis=-1)  # [R, E]
    post_rev = np.ascontiguousarray(pos_emb[::-1, :].T).astype(ml_dtypes.bfloat16)
    ident = np.eye(128, dtype=ml_dtypes.bfloat16)
    onescol = np.ones((1, 128), dtype=ml_dtypes.bfloat16)
    return post_rev, ident, onescol


def kernel(values, keys, query, mask, Wv, Wk, Wq, Wr, u_bias, v_bias, Wo, bo):
    values = np.ascontiguousarray(np.asarray(values, dtype=np.float32))
    keys = np.ascontiguousarray(np.asarray(keys, dtype=np.float32))
    query = np.ascontiguousarray(np.asarray(query, dtype=np.float32))
    mask = np.ascontiguousarray(np.asarray(mask, dtype=np.int32))
    Wv = np.ascontiguousarray(np.asarray(Wv, dtype=np.float32))
    Wk = np.ascontiguousarray(np.asarray(Wk, dtype=np.float32))
    Wq = np.ascontiguousarray(np.asarray(Wq, dtype=np.float32))
    Wr = np.ascontiguousarray(np.asarray(Wr, dtype=np.float32))
    Wo = np.ascontiguousarray(np.asarray(Wo, dtype=np.float32))
    u_flat = np.ascontiguousarray(np.asarray(u_bias, dtype=np.float32).reshape(1, EMBED))
    v_flat = np.ascontiguousarray(np.asarray(v_bias, dtype=np.float32).reshape(1, EMBED))
    bo_flat = np.ascontiguousarray(np.asarray(bo, dtype=np.float32).reshape(1, EMBED))

    post_rev, ident, onescol = _host_constants()

    masked = bool((mask == 0).any())
    nc = build_program(masked=masked)

    in_maps = []
    for c in range(NCORES):
        s = slice(NL * c, NL * (c + 1))
        in_maps.append({
            "q_in": query[s], "k_in": keys[s], "v_in": values[s],
            **({"m_in": mask[s]} if masked else {}),
            "wq_in": Wq, "wk_in": Wk, "wv_in": Wv, "wr_in": Wr, "wo_in": Wo,
            "u_in": u_flat, "vb_in": v_flat, "bo_in": bo_flat,
            "post_in": post_rev, "id_in": ident, "onescol_in": onescol,
        })

    res = run_bass_kernel_spmd(nc, in_maps, core_ids=list(range(NCORES)))
    global LAST_RESULTS
    LAST_RESULTS = res

    out = np.empty((NB, QL, EMBED), dtype=np.float32)
    attn = np.empty((NB, HEADS, QL, KL), dtype=np.float32)
    for c in range(NCORES):
        r = res.results[c]
        out[NL * c:NL * (c + 1)] = r["out_o"]
        attn[NL * c:NL * (c + 1)] = np.asarray(r["attn_o"], dtype=np.float32)
    return out, attn


# revision 23
# speedup vs baseline: 2.5492x; 1.1459x over previous
"""Transformer-XL style MultiHeadAttention on 8 TRN2 NeuronCores.

Data-parallel over batch N=16 -> 2 batches per core. Full computation on
device per core:
  - per-head projections q/k/v (shared 64x64 weights, block-diag over head
    pairs on the 128-wide PE contraction)
  - relative key table rel = pos_emb @ Wr.T computed on device (pos_emb is an
    input-independent sinusoid constant, host-precomputed, fed reversed and
    transposed so the Toeplitz shift becomes a forward-strided read)
  - energy = (q+u)k^T + (q+v)rel^T(shifted) + mask bias, softmax, attn@v,
    output projection with Wo/bo
  - the relative-position shift (energy[q,k] takes rel index q-k+512) is done
    by a DRAM roundtrip: P[128,4,640](bf16) -> one diagonal-strided 3D DMA
    read (row stride 639 elements) -> [128,4,512] aligned tiles ->
    identity-matmul accumulate into the content PSUM.

Matmuls run in bf16 (f32 PSUM accumulation); softmax in f32 on ACT with
fused row-sum (accum_out). Input/weight transposes run on the PE
(transpose mode); the attention transpose for attn@v reuses the attention
DRAM output via xbar DMA-transpose loads.
"""

import numpy as np
import ml_dtypes

import concourse.bass as bass
import concourse.mybir as mybir
import concourse.tile as tile
from concourse import bacc
from concourse.bass_utils import run_bass_kernel_spmd

F32 = mybir.dt.float32
BF16 = mybir.dt.bfloat16
I32 = mybir.dt.int32

EMBED = 512
HEADS = 8
HEAD_D = 64
NB = 16          # full batch
NL = 2           # batches per core
QL = 512
KL = 512
R = QL + KL      # 1024 relative positions
NCORES = 8
HP = HEADS // 2  # head pairs

LAST_RESULTS = None
USE_POOL_NORM = False
AF = mybir.ActivationFunctionType
ALU = mybir.AluOpType


def build_program(masked=True):
    nc = bacc.Bacc("TRN2", target_bir_lowering=False, debug=False)

    # ---- I/O ----
    q_in = nc.dram_tensor("q_in", [NL, QL, EMBED], F32, kind="ExternalInput")
    k_in = nc.dram_tensor("k_in", [NL, KL, EMBED], F32, kind="ExternalInput")
    v_in = nc.dram_tensor("v_in", [NL, KL, EMBED], F32, kind="ExternalInput")
    m_in = (nc.dram_tensor("m_in", [NL, QL, KL], I32, kind="ExternalInput")
            if masked else None)
    wq_in = nc.dram_tensor("wq_in", [HEAD_D, HEAD_D], F32, kind="ExternalInput")
    wk_in = nc.dram_tensor("wk_in", [HEAD_D, HEAD_D], F32, kind="ExternalInput")
    wv_in = nc.dram_tensor("wv_in", [HEAD_D, HEAD_D], F32, kind="ExternalInput")
    wr_in = nc.dram_tensor("wr_in", [EMBED, EMBED], F32, kind="ExternalInput")
    wo_in = nc.dram_tensor("wo_in", [EMBED, EMBED], F32, kind="ExternalInput")
    u_in = nc.dram_tensor("u_in", [1, EMBED], F32, kind="ExternalInput")
    vb_in = nc.dram_tensor("vb_in", [1, EMBED], F32, kind="ExternalInput")
    bo_in = nc.dram_tensor("bo_in", [1, EMBED], F32, kind="ExternalInput")
    post_in = nc.dram_tensor("post_in", [EMBED, R], BF16, kind="ExternalInput")
    id_in = nc.dram_tensor("id_in", [128, 128], BF16, kind="ExternalInput")
    onescol_in = nc.dram_tensor("onescol_in", [1, 128], BF16, kind="ExternalInput")

    attn_o = nc.dram_tensor("attn_o", [NL, HEADS, QL, KL], BF16,
                            kind="ExternalOutput")
    out_o = nc.dram_tensor("out_o", [NL, QL, EMBED], F32, kind="ExternalOutput")

    with tile.TileContext(nc) as tc:
        with tc.tile_pool(name="persist", bufs=1) as persist, \
             tc.tile_pool(name="slab", bufs=1) as slab, \
             tc.tile_pool(name="proj", bufs=2) as proj, \
             tc.tile_pool(name="hdat", bufs=4) as hdat, \
             tc.tile_pool(name="setuppool", bufs=2) as setuppool, \
             tc.tile_pool(name="small", bufs=4) as small, \
             tc.tile_pool(name="pdpool", bufs=3) as pdpool, \
             tc.tile_pool(name="pospool", bufs=7) as pospool, \
             tc.tile_pool(name="outp", bufs=2) as outp, \
             tc.tile_pool(name="pp", bufs=2, space="PSUM") as pp, \
             tc.tile_pool(name="cc", bufs=2, space="PSUM") as cc, \
             tc.tile_pool(name="mm", bufs=2, space="PSUM") as mm, \
             tc.tile_pool(name="dscratch", bufs=8, space="DRAM") as dscratch:

            # ================= SETUP =================
            ident = persist.tile([128, 128], BF16)
            nc.sync.dma_start(out=ident[:], in_=id_in[:])
            onescol = persist.tile([1, 128], BF16)
            nc.sync.dma_start(out=onescol[:], in_=onescol_in[:])

            post = persist.tile([128, 4, R], BF16)  # [e within tile, etile, r]
            nc.sync.dma_start(
                out=post[:],
                in_=post_in[:].rearrange("(et e) r -> e et r", e=128))

            # per-partition bias vectors for head pairs: [128, HP]
            u_sb = persist.tile([128, HP], F32)
            vb_sb = persist.tile([128, HP], F32)
            for hp in range(HP):
                nc.sync.dma_start(out=u_sb[:, hp:hp + 1],
                                  in_=u_in[0, 128 * hp:128 * hp + 128])
                nc.sync.dma_start(out=vb_sb[:, hp:hp + 1],
                                  in_=vb_in[0, 128 * hp:128 * hp + 128])
            bo_b = persist.tile([1, EMBED], BF16)
            bo_f = setuppool.tile([1, EMBED], F32, tag="bof")
            nc.sync.dma_start(out=bo_f[:], in_=bo_in[:])
            nc.vector.tensor_copy(bo_b[:], bo_f[:])

            # f32 identity for PE transposes of f32 data
            idf = persist.tile([128, 128], F32)
            nc.vector.tensor_copy(idf[:], ident[:])

            # small weight transposes via PE transpose-mode
            bds = {}
            for name, w_dram in (("q", wq_in), ("k", wk_in), ("v", wv_in)):
                w_f = setuppool.tile([64, 64], F32, tag="w_f")
                nc.sync.dma_start(out=w_f[:], in_=w_dram[:])
                wtp = mm.tile([64, 64], F32, tag="mmt")
                nc.tensor.transpose(wtp[:], w_f[:], idf[0:64, 0:64])
                bd = persist.tile([128, 128], BF16, tag="bd" + name)
                nc.vector.memset(bd[:], 0.0)
                nc.vector.tensor_copy(bd[0:64, 0:64], wtp[:])
                nc.vector.tensor_copy(bd[64:128, 64:128], wtp[:])
                bds[name] = bd

            # Wr^T via PE transpose: wrt[e(part), e-tile?, e'] ; note
            # wrt[:, it, ot-block] comes from source rows ot, cols it.
            wrt = persist.tile([128, 4, EMBED], BF16, tag="wrt")
            for ot in range(4):
                wrow_f = setuppool.tile([128, EMBED], F32, tag="wrow_f")
                nc.sync.dma_start(out=wrow_f[:],
                                  in_=wr_in[128 * ot:128 * ot + 128, :])
                wtp = mm.tile([128, 512], F32, tag="mmt")
                for it in range(4):
                    nc.tensor.transpose(wtp[:, 128 * it:128 * it + 128],
                                        wrow_f[:, 128 * it:128 * it + 128],
                                        idf[:])
                for it in range(4):
                    nc.vector.tensor_copy(wrt[:, it, 128 * ot:128 * ot + 128],
                                          wtp[:, 128 * it:128 * it + 128])
                # rel for e'-tile et=ot only needs wrt[:, :, ot-block]
                # -> emit rel(ot) right away (pipelines with next wrow load)

            # rel table: rel_sb[e'g, e't, jcol] = rel[1023-jcol, e'] (bf16)
            rel_sb = persist.tile([128, 4, R], BF16, tag="rel")
            for et in range(4):
                for rh in range(2):
                    rpsum = mm.tile([128, 512], F32, tag="mmt")
                    for e in range(4):
                        nc.tensor.matmul(
                            rpsum[:],
                            wrt[:, e, 128 * et:128 * et + 128],
                            post[:, e, 512 * rh:512 * rh + 512],
                            start=(e == 0), stop=(e == 3))
                    nc.scalar.activation(
                        rel_sb[:, et, 512 * rh:512 * rh + 512], rpsum[:],
                        AF.Identity)

            # Wo^T prepared the same way (consumed only at first emit_E)
            wot = persist.tile([128, 4, EMBED], BF16, tag="wot")

            def emit_wot():
                for ot in range(4):
                    wrow_f = setuppool.tile([128, EMBED], F32, tag="wrow_f")
                    nc.sync.dma_start(out=wrow_f[:],
                                      in_=wo_in[128 * ot:128 * ot + 128, :])
                    wtp = mm.tile([128, 512], F32, tag="mmt")
                    for it in range(4):
                        nc.tensor.transpose(wtp[:, 128 * it:128 * it + 128],
                                            wrow_f[:, 128 * it:128 * it + 128],
                                            idf[:])
                    for it in range(4):
                        nc.vector.tensor_copy(
                            wot[:, it, 128 * ot:128 * ot + 128],
                            wtp[:, 128 * it:128 * it + 128])

            # mask bias: (mask-1)*1e20 as bf16, [128, NL*4, 512]
            maskb = None
            if masked:
                maskb = persist.tile([128, NL * 4, 512], BF16, tag="maskb")
                for n in range(NL):
                    mrow = setuppool.tile([128, 4, 512], I32, tag="mrow")
                    nc.sync.dma_start(
                        out=mrow[:],
                        in_=m_in[n].rearrange("(qt p) k -> p qt k", p=128))
                    for qt in range(4):
                        nc.vector.tensor_scalar(
                            out=maskb[:, 4 * n + qt, :], in0=mrow[:, qt, :],
                            scalar1=-1.0, scalar2=1.0e20,
                            op0=ALU.add, op1=ALU.mult)

            # ============ per-batch state build ============
            def emit_slabs(n):
                """load x f32, SWDGE cast-write bf16 to DRAM, xbar-load
                transposed -> xt dict [e, etile, tok]"""
                out = {}
                for name, src in (("q", q_in), ("k", k_in), ("v", v_in)):
                    x_f = slab.tile([128, 4, EMBED], F32, tag="x_f")
                    nc.gpsimd.dma_start(
                        out=x_f[:],
                        in_=src[n].rearrange("(tt t) e -> t tt e", t=128))
                    xscr = dscratch.tile([QL, EMBED], BF16, tag="xscr")
                    nc.gpsimd.dma_start(
                        out=xscr[:].rearrange("(tt p) e -> p tt e", p=128),
                        in_=x_f[:])
                    xt = slab.tile([128, 4, QL], BF16, tag="xt_" + name)
                    nc.sync.dma_start_transpose(out=xt[:], in_=xscr[:])
                    out[name] = xt
                return out

            def emit_proj(n, xt):
                """projections for batch n -> (qu, qv, kt, v) tiles"""
                qu = proj.tile([128, HP, QL], BF16, tag="qu")
                qv = proj.tile([128, HP, QL], BF16, tag="qv")
                kt = proj.tile([128, HP, KL], BF16, tag="kt")
                vsb = proj.tile([128, 4, EMBED], BF16, tag="vsb")
                for hp in range(HP):
                    qp = mm.tile([128, QL], F32, tag="mmt")
                    nc.tensor.matmul(qp[:], bds["q"], xt["q"][:, hp, :],
                                     start=True, stop=True)
                    nc.vector.tensor_scalar(out=qu[:, hp, :], in0=qp[:],
                                            scalar1=u_sb[:, hp:hp + 1],
                                            scalar2=None, op0=ALU.add)
                    nc.vector.tensor_scalar(out=qv[:, hp, :], in0=qp[:],
                                            scalar1=vb_sb[:, hp:hp + 1],
                                            scalar2=None, op0=ALU.add)
                    kp = mm.tile([128, KL], F32, tag="mmt")
                    nc.tensor.matmul(kp[:], bds["k"], xt["k"][:, hp, :],
                                     start=True, stop=True)
                    nc.vector.tensor_copy(kt[:, hp, :], kp[:])
                    vp = mm.tile([128, 512], F32, tag="mmt")
                    for tt in range(4):
                        nc.tensor.matmul(vp[:, 128 * tt:128 * tt + 128],
                                         xt["v"][:, hp, 128 * tt:128 * tt + 128],
                                         bds["v"], start=True, stop=True)
                    for tt in range(4):
                        nc.vector.tensor_copy(
                            vsb[:, tt, 128 * hp:128 * hp + 128],
                            vp[:, 128 * tt:128 * tt + 128])
                return qu, qv, kt, vsb

            # ============ streamed attention ============
            def head_slice(t, h, qt=None):
                base = (h % 2) * 64
                if qt is None:
                    return t[base:base + 64, h // 2, :]
                return t[base:base + 64, h // 2, 128 * qt:128 * qt + 128]

            def emit_P(n, h, st):
                """position matmuls + drains + scratch write + diag read"""
                qv = st["proj"][1]
                rel_h = rel_sb[(h % 2) * 64:(h % 2) * 64 + 64, h // 2, :]
                pd4 = pdpool.tile([128, 4, 640], BF16, tag="pd4")
                for qt in range(4):
                    ws = 384 - 128 * qt
                    P = pp.tile([128, 640], F32, tag="pp")
                    nc.tensor.matmul(P[:, 0:512], head_slice(qv, h, qt),
                                     rel_h[:, ws:ws + 512], start=True, stop=True)
                    nc.tensor.matmul(P[:, 512:640], head_slice(qv, h, qt),
                                     rel_h[:, ws + 512:ws + 640],
                                     start=True, stop=True)
                    nc.vector.tensor_copy(pd4[:, qt, :], P[:])
                scr = dscratch.tile([4, 128, 640], BF16, tag="scr")
                # write [p, qt, j] -> [qt, p, j]
                nc.sync.dma_start(
                    out=scr[:].rearrange("qt p j -> p qt j"), in_=pd4[:])
                # diagonal read: pos[p, qt, k] = scr[qt, p, 127 - p + k]
                pos4 = pospool.tile([128, 4, 512], BF16, tag="pos4")
                dsrc = bass.AP(scr[:].tensor, 127,
                               [[639, 128], [128 * 640, 4], [1, 512]])
                nc.sync.dma_start(out=pos4[:], in_=dsrc)
                st["pos4"] = pos4

            def emit_C(n, h, st):
                qu, _, kt, _ = st["proj"]
                st["Z"] = small.tile([128, 4], F32, tag="Z", name="zt")
                st["expm"] = hdat.tile([128, 4, 512], BF16, tag="expm",
                                       name="expm")
                for qt in range(4):
                    C = cc.tile([128, 512], F32, tag="cc")
                    nc.tensor.matmul(C[:], head_slice(qu, h, qt),
                                     kt[(h % 2) * 64:(h % 2) * 64 + 64, h // 2, :],
                                     start=True, stop=False)
                    if masked:
                        nc.tensor.matmul(C[:], ident[:], maskb[:, 4 * n + qt, :],
                                         start=False, stop=False)
                    nc.tensor.matmul(C[:], ident[:], st["pos4"][:, qt, :],
                                     start=False, stop=True)
                    nc.scalar.activation(st["expm"][:, qt, :], C[:], AF.Exp,
                                         scale=0.125,
                                         accum_out=st["Z"][:, qt:qt + 1])


            def emit_F1(n, h, st):
                rc = small.tile([128, 4], F32, tag="rc")
                nc.vector.reciprocal(rc[:], st["Z"][:])
                attn_n = st["expm"]
                for qt in range(4):
                    nc.vector.tensor_scalar(
                        out=attn_n[:, qt, :], in0=st["expm"][:, qt, :],
                        scalar1=rc[:, qt:qt + 1], scalar2=None, op0=ALU.mult)
                nc.gpsimd.dma_start(
                    out=attn_o[n, h].rearrange("(qt p) k -> p qt k", p=128),
                    in_=attn_n[:])

            def emit_F2(n, h, st):
                attn_t = hdat.tile([128, 4, 512], BF16, tag="attn_t")
                nc.scalar.dma_start_transpose(out=attn_t[:], in_=attn_o[n, h][:])
                vsb = st["proj"][3]
                ctxp = mm.tile([64, 512], F32, tag="mmt")
                for ktile in range(4):
                    nc.tensor.matmul(
                        ctxp[:],
                        vsb[:, ktile, 64 * h:64 * h + 64],
                        attn_t[:, ktile, :],
                        start=(ktile == 0), stop=(ktile == 3))
                nc.vector.tensor_copy(
                    st["ctx"][(h % 2) * 64:(h % 2) * 64 + 64, h // 2, :],
                    ctxp[:])

            def emit_E(n, st):
                osb = outp.tile([128, 4, 512], F32, tag="osb")
                for tt in range(4):
                    op = mm.tile([128, 512], F32, tag="mmt")
                    for et in range(4):
                        nc.tensor.matmul(
                            op[:], st["ctx"][:, et, 128 * tt:128 * tt + 128],
                            wot[:, et, :], start=(et == 0), stop=False)
                    nc.tensor.matmul(op[:], onescol[:], bo_b[:],
                                     start=False, stop=True)
                    nc.vector.tensor_copy(osb[:, tt, :], op[:])
                nc.gpsimd.dma_start(
                    out=out_o[n].rearrange("(tt p) e -> p tt e", p=128),
                    in_=osb[:])

            # ---- main stream over head PAIRS (dense PE bursts for HAM) ----
            PSKEW = 3  # pairs between P and C phases
            DF1 = 1    # pairs between C and F1
            DF2 = 2    # pairs between C and F2
            pairs = [(n, h) for n in range(NL) for h in range(0, HEADS, 2)]
            state = {}
            f1q, f2q = [], []
            cur_n = -1
            wot_done = [False]

            def do_F2(fn, fh, st):
                emit_F2(fn, fh, st)
                if fh == HEADS - 1:
                    emit_E(fn, state[(fn, fh)])

            def advance(do_c=None):
                if do_c is not None:
                    jn, jh = do_c
                    emit_C(jn, jh, state[(jn, jh)])
                    emit_C(jn, jh + 1, state[(jn, jh + 1)])
                    f1q.append((jn, jh))
                    f2q.append((jn, jh))
                while len(f1q) > DF1:
                    fn, fh = f1q.pop(0)
                    emit_F1(fn, fh, state[(fn, fh)])
                    emit_F1(fn, fh + 1, state[(fn, fh + 1)])
                while len(f2q) > DF2:
                    fn, fh = f2q.pop(0)
                    do_F2(fn, fh, state[(fn, fh)])
                    do_F2(fn, fh + 1, state[(fn, fh + 1)])

            for i, (n, h) in enumerate(pairs):
                if n != cur_n:
                    cur_n = n
                    xt = emit_slabs(n)
                    projt = emit_proj(n, xt)
                    ctx_sb = proj.tile([128, HP, QL], BF16, tag="ctx",
                                       name="ctx_sb")
                    nstate = {"proj": projt, "ctx": ctx_sb}
                for hh in (h, h + 1):
                    state[(n, hh)] = dict(nstate)
                    emit_P(n, hh, state[(n, hh)])
                if i == 1 and not wot_done[0]:
                    wot_done[0] = True
                    emit_wot()
                j = i - PSKEW
                if j >= 0:
                    advance(do_c=pairs[j])
            for j in range(len(pairs) - PSKEW, len(pairs)):
                advance(do_c=pairs[j])
            while f1q:
                fn, fh = f1q.pop(0)
                emit_F1(fn, fh, state[(fn, fh)])
                emit_F1(fn, fh + 1, state[(fn, fh + 1)])
            while f2q:
                fn, fh = f2q.pop(0)
                do_F2(fn, fh, state[(fn, fh)])
                do_F2(fn, fh + 1, state[(fn, fh + 1)])

    nc.compile()
    return nc


def _host_constants():
    invf = (10000.0 ** (-np.arange(0, EMBED, 2, dtype=np.float64) / EMBED))
    seq = np.arange(R, dtype=np.float64)[:, None] * invf[None, :]
    pos_emb = np.concatenate([np.sin(seq), np.cos(seq)], axis=-1)  # [R, E]
    post_rev = np.ascontiguousarray(pos_emb[::-1, :].T).astype(ml_dtypes.bfloat16)
    ident = np.eye(128, dtype=ml_dtypes.bfloat16)
    onescol = np.ones((1, 128), dtype=ml_dtypes.bfloat16)
    return post_rev, ident, onescol


def kernel(values, keys, query, mask, Wv, Wk, Wq, Wr, u_bias, v_bias, Wo, bo):
    values = np.ascontiguousarray(np.asarray(values, dtype=np.float32))
    keys = np.ascontiguousarray(np.asarray(keys, dtype=np.float32))
    query = np.ascontiguousarray(np.asarray(query, dtype=np.float32))
    mask = np.ascontiguousarray(np.asarray(mask, dtype=np.int32))
    Wv = np.ascontiguousarray(np.asarray(Wv, dtype=np.float32))
    Wk = np.ascontiguousarray(np.asarray(Wk, dtype=np.float32))
    Wq = np.ascontiguousarray(np.asarray(Wq, dtype=np.float32))
    Wr = np.ascontiguousarray(np.asarray(Wr, dtype=np.float32))
    Wo = np.ascontiguousarray(np.asarray(Wo, dtype=np.float32))
    u_flat = np.ascontiguousarray(np.asarray(u_bias, dtype=np.float32).reshape(1, EMBED))
    v_flat = np.ascontiguousarray(np.asarray(v_bias, dtype=np.float32).reshape(1, EMBED))
    bo_flat = np.ascontiguousarray(np.asarray(bo, dtype=np.float32).reshape(1, EMBED))

    post_rev, ident, onescol = _host_constants()

    masked = bool((mask == 0).any())
    nc = build_program(masked=masked)

    in_maps = []
    for c in range(NCORES):
        s = slice(NL * c, NL * (c + 1))
        in_maps.append({
            "q_in": query[s], "k_in": keys[s], "v_in": values[s],
            **({"m_in": mask[s]} if masked else {}),
            "wq_in": Wq, "wk_in": Wk, "wv_in": Wv, "wr_in": Wr, "wo_in": Wo,
            "u_in": u_flat, "vb_in": v_flat, "bo_in": bo_flat,
            "post_in": post_rev, "id_in": ident, "onescol_in": onescol,
        })

    res = run_bass_kernel_spmd(nc, in_maps, core_ids=list(range(NCORES)))
    global LAST_RESULTS
    LAST_RESULTS = res

    out = np.empty((NB, QL, EMBED), dtype=np.float32)
    attn = np.empty((NB, HEADS, QL, KL), dtype=np.float32)
    for c in range(NCORES):
        r = res.results[c]
        out[NL * c:NL * (c + 1)] = r["out_o"]
        attn[NL * c:NL * (c + 1)] = np.asarray(r["attn_o"], dtype=np.float32)
    return out, attn


# revision 24
# speedup vs baseline: 2.9003x; 1.1377x over previous
"""Transformer-XL style MultiHeadAttention on 8 TRN2 NeuronCores.

Data-parallel over batch N=16 -> 2 batches per core. Full computation on
device per core:
  - per-head projections q/k/v (shared 64x64 weights, block-diag over head
    pairs on the 128-wide PE contraction)
  - relative key table rel = pos_emb @ Wr.T computed on device (pos_emb is an
    input-independent sinusoid constant, host-precomputed, fed reversed and
    transposed so the Toeplitz shift becomes a forward-strided read)
  - energy = (q+u)k^T + (q+v)rel^T(shifted) + mask bias, softmax, attn@v,
    output projection with Wo/bo
  - the relative-position shift (energy[q,k] takes rel index q-k+512) is done
    by a DRAM roundtrip: P[128,4,640](bf16) -> one diagonal-strided 3D DMA
    read (row stride 639 elements) -> [128,4,512] aligned tiles ->
    identity-matmul accumulate into the content PSUM.

Matmuls run in bf16 (f32 PSUM accumulation); softmax in f32 on ACT with
fused row-sum (accum_out). Input/weight transposes run on the PE
(transpose mode); the attention transpose for attn@v reuses the attention
DRAM output via xbar DMA-transpose loads.
"""

import numpy as np
import ml_dtypes

import concourse.bass as bass
import concourse.mybir as mybir
import concourse.tile as tile
from concourse import bacc
from concourse.bass_utils import run_bass_kernel_spmd

F32 = mybir.dt.float32
BF16 = mybir.dt.bfloat16
I32 = mybir.dt.int32

EMBED = 512
HEADS = 8
HEAD_D = 64
NB = 16          # full batch
NL = 2           # batches per core
QL = 512
KL = 512
R = QL + KL      # 1024 relative positions
NCORES = 8
HP = HEADS // 2  # head pairs

LAST_RESULTS = None
USE_POOL_NORM = False
AF = mybir.ActivationFunctionType
ALU = mybir.AluOpType


def build_program(masked=True):
    nc = bacc.Bacc("TRN2", target_bir_lowering=False, debug=False)

    # ---- I/O ----
    q_in = nc.dram_tensor("q_in", [NL, QL, EMBED], F32, kind="ExternalInput")
    k_in = nc.dram_tensor("k_in", [NL, KL, EMBED], F32, kind="ExternalInput")
    v_in = nc.dram_tensor("v_in", [NL, KL, EMBED], F32, kind="ExternalInput")
    m_in = (nc.dram_tensor("m_in", [NL, QL, KL], I32, kind="ExternalInput")
            if masked else None)
    wq_in = nc.dram_tensor("wq_in", [HEAD_D, HEAD_D], F32, kind="ExternalInput")
    wk_in = nc.dram_tensor("wk_in", [HEAD_D, HEAD_D], F32, kind="ExternalInput")
    wv_in = nc.dram_tensor("wv_in", [HEAD_D, HEAD_D], F32, kind="ExternalInput")
    wr_in = nc.dram_tensor("wr_in", [EMBED, EMBED], F32, kind="ExternalInput")
    wo_in = nc.dram_tensor("wo_in", [EMBED, EMBED], F32, kind="ExternalInput")
    u_in = nc.dram_tensor("u_in", [1, EMBED], F32, kind="ExternalInput")
    vb_in = nc.dram_tensor("vb_in", [1, EMBED], F32, kind="ExternalInput")
    bo_in = nc.dram_tensor("bo_in", [1, EMBED], F32, kind="ExternalInput")
    post_in = nc.dram_tensor("post_in", [EMBED, R], BF16, kind="ExternalInput")
    id_in = nc.dram_tensor("id_in", [128, 128], BF16, kind="ExternalInput")
    onescol_in = nc.dram_tensor("onescol_in", [1, 128], BF16, kind="ExternalInput")

    attn_o = nc.dram_tensor("attn_o", [NL, HEADS, QL, KL], BF16,
                            kind="ExternalOutput")
    out_o = nc.dram_tensor("out_o", [NL, QL, EMBED], F32, kind="ExternalOutput")

    with tile.TileContext(nc) as tc:
        with tc.tile_pool(name="persist", bufs=1) as persist, \
             tc.tile_pool(name="slab", bufs=1) as slab, \
             tc.tile_pool(name="proj", bufs=2) as proj, \
             tc.tile_pool(name="hdat", bufs=4) as hdat, \
             tc.tile_pool(name="setuppool", bufs=2) as setuppool, \
             tc.tile_pool(name="small", bufs=4) as small, \
             tc.tile_pool(name="pdpool", bufs=3) as pdpool, \
             tc.tile_pool(name="pospool", bufs=7) as pospool, \
             tc.tile_pool(name="outp", bufs=2) as outp, \
             tc.tile_pool(name="pp", bufs=2, space="PSUM") as pp, \
             tc.tile_pool(name="cc", bufs=2, space="PSUM") as cc, \
             tc.tile_pool(name="mm", bufs=2, space="PSUM") as mm, \
             tc.tile_pool(name="dscratch", bufs=8, space="DRAM") as dscratch:

            # ================= SETUP =================
            ident = persist.tile([128, 128], BF16)
            nc.sync.dma_start(out=ident[:], in_=id_in[:])
            onescol = persist.tile([1, 128], BF16)
            nc.sync.dma_start(out=onescol[:], in_=onescol_in[:])

            post = persist.tile([128, 4, R], BF16)  # [e within tile, etile, r]
            nc.sync.dma_start(
                out=post[:],
                in_=post_in[:].rearrange("(et e) r -> e et r", e=128))

            # per-partition bias vectors for head pairs: [128, HP]
            u_sb = persist.tile([128, HP], F32)
            vb_sb = persist.tile([128, HP], F32)
            for hp in range(HP):
                nc.sync.dma_start(out=u_sb[:, hp:hp + 1],
                                  in_=u_in[0, 128 * hp:128 * hp + 128])
                nc.sync.dma_start(out=vb_sb[:, hp:hp + 1],
                                  in_=vb_in[0, 128 * hp:128 * hp + 128])
            bo_b = persist.tile([1, EMBED], BF16)
            bo_f = setuppool.tile([1, EMBED], F32, tag="bof")
            nc.sync.dma_start(out=bo_f[:], in_=bo_in[:])
            nc.vector.tensor_copy(bo_b[:], bo_f[:])

            # f32 identity for PE transposes of f32 data
            idf = persist.tile([128, 128], F32)
            nc.vector.tensor_copy(idf[:], ident[:])

            # small weight transposes via PE transpose-mode
            bds = {}
            for name, w_dram in (("q", wq_in), ("k", wk_in), ("v", wv_in)):
                w_f = setuppool.tile([64, 64], F32, tag="w_f")
                nc.sync.dma_start(out=w_f[:], in_=w_dram[:])
                wtp = mm.tile([64, 64], F32, tag="mmt")
                nc.tensor.transpose(wtp[:], w_f[:], idf[0:64, 0:64])
                bd = persist.tile([128, 128], BF16, tag="bd" + name)
                nc.vector.memset(bd[:], 0.0)
                nc.vector.tensor_copy(bd[0:64, 0:64], wtp[:])
                nc.vector.tensor_copy(bd[64:128, 64:128], wtp[:])
                bds[name] = bd

            # Wr^T via PE transpose: wrt[e(part), e-tile?, e'] ; note
            # wrt[:, it, ot-block] comes from source rows ot, cols it.
            wrt = persist.tile([128, 4, EMBED], BF16, tag="wrt")
            for ot in range(4):
                wrow_f = setuppool.tile([128, EMBED], F32, tag="wrow_f")
                nc.sync.dma_start(out=wrow_f[:],
                                  in_=wr_in[128 * ot:128 * ot + 128, :])
                wtp = mm.tile([128, 512], F32, tag="mmt")
                for it in range(4):
                    nc.tensor.transpose(wtp[:, 128 * it:128 * it + 128],
                                        wrow_f[:, 128 * it:128 * it + 128],
                                        idf[:])
                for it in range(4):
                    nc.vector.tensor_copy(wrt[:, it, 128 * ot:128 * ot + 128],
                                          wtp[:, 128 * it:128 * it + 128])
                # rel for e'-tile et=ot only needs wrt[:, :, ot-block]
                # -> emit rel(ot) right away (pipelines with next wrow load)

            # rel table: rel_sb[e'g, e't, jcol] = rel[1023-jcol, e'] (bf16)
            rel_sb = persist.tile([128, 4, R], BF16, tag="rel")
            for et in range(4):
                for rh in range(2):
                    rpsum = mm.tile([128, 512], F32, tag="mmt")
                    for e in range(4):
                        nc.tensor.matmul(
                            rpsum[:],
                            wrt[:, e, 128 * et:128 * et + 128],
                            post[:, e, 512 * rh:512 * rh + 512],
                            start=(e == 0), stop=(e == 3))
                    nc.scalar.activation(
                        rel_sb[:, et, 512 * rh:512 * rh + 512], rpsum[:],
                        AF.Identity)

            # Wo^T prepared the same way (consumed only at first emit_E)
            wot = persist.tile([128, 4, EMBED], BF16, tag="wot")

            def emit_wot():
                for ot in range(4):
                    wrow_f = setuppool.tile([128, EMBED], F32, tag="wrow_f")
                    nc.sync.dma_start(out=wrow_f[:],
                                      in_=wo_in[128 * ot:128 * ot + 128, :])
                    wtp = mm.tile([128, 512], F32, tag="mmt")
                    for it in range(4):
                        nc.tensor.transpose(wtp[:, 128 * it:128 * it + 128],
                                            wrow_f[:, 128 * it:128 * it + 128],
                                            idf[:])
                    for it in range(4):
                        nc.vector.tensor_copy(
                            wot[:, it, 128 * ot:128 * ot + 128],
                            wtp[:, 128 * it:128 * it + 128])

            # mask bias: (mask-1)*1e20 as bf16, [128, NL*4, 512]
            maskb = None
            if masked:
                maskb = persist.tile([128, NL * 4, 512], BF16, tag="maskb")
                for n in range(NL):
                    mrow = setuppool.tile([128, 4, 512], I32, tag="mrow")
                    nc.sync.dma_start(
                        out=mrow[:],
                        in_=m_in[n].rearrange("(qt p) k -> p qt k", p=128))
                    for qt in range(4):
                        nc.vector.tensor_scalar(
                            out=maskb[:, 4 * n + qt, :], in0=mrow[:, qt, :],
                            scalar1=-1.0, scalar2=1.0e20,
                            op0=ALU.add, op1=ALU.mult)

            # ============ per-batch state build ============
            def emit_slabs(n):
                """load x f32, SWDGE cast-write bf16 to DRAM, xbar-load
                transposed -> xt dict [e, etile, tok]"""
                out = {}
                for name, src in (("q", q_in), ("k", k_in), ("v", v_in)):
                    x_f = slab.tile([128, 4, EMBED], F32, tag="x_f")
                    nc.gpsimd.dma_start(
                        out=x_f[:],
                        in_=src[n].rearrange("(tt t) e -> t tt e", t=128))
                    xscr = dscratch.tile([QL, EMBED], BF16, tag="xscr")
                    nc.gpsimd.dma_start(
                        out=xscr[:].rearrange("(tt p) e -> p tt e", p=128),
                        in_=x_f[:])
                    xt = slab.tile([128, 4, QL], BF16, tag="xt_" + name)
                    nc.sync.dma_start_transpose(out=xt[:], in_=xscr[:])
                    out[name] = xt
                return out

            def emit_proj(n, xt):
                """projections for batch n -> (qu, qv, kt, v) tiles"""
                qu = proj.tile([128, HP, QL], BF16, tag="qu")
                qv = proj.tile([128, HP, QL], BF16, tag="qv")
                kt = proj.tile([128, HP, KL], BF16, tag="kt")
                vsb = proj.tile([128, 4, EMBED], BF16, tag="vsb")
                for hp in range(HP):
                    qp = mm.tile([128, QL], F32, tag="mmt")
                    nc.tensor.matmul(qp[:], bds["q"], xt["q"][:, hp, :],
                                     start=True, stop=True)
                    nc.vector.tensor_scalar(out=qu[:, hp, :], in0=qp[:],
                                            scalar1=u_sb[:, hp:hp + 1],
                                            scalar2=None, op0=ALU.add)
                    nc.vector.tensor_scalar(out=qv[:, hp, :], in0=qp[:],
                                            scalar1=vb_sb[:, hp:hp + 1],
                                            scalar2=None, op0=ALU.add)
                    kp = mm.tile([128, KL], F32, tag="mmt")
                    nc.tensor.matmul(kp[:], bds["k"], xt["k"][:, hp, :],
                                     start=True, stop=True)
                    nc.vector.tensor_copy(kt[:, hp, :], kp[:])
                    vp = mm.tile([128, 512], F32, tag="mmt")
                    for tt in range(4):
                        nc.tensor.matmul(vp[:, 128 * tt:128 * tt + 128],
                                         xt["v"][:, hp, 128 * tt:128 * tt + 128],
                                         bds["v"], start=True, stop=True)
                    for tt in range(4):
                        nc.vector.tensor_copy(
                            vsb[:, tt, 128 * hp:128 * hp + 128],
                            vp[:, 128 * tt:128 * tt + 128])
                return qu, qv, kt, vsb

            # ============ streamed attention ============
            def head_slice(t, h, qt=None):
                base = (h % 2) * 64
                if qt is None:
                    return t[base:base + 64, h // 2, :]
                return t[base:base + 64, h // 2, 128 * qt:128 * qt + 128]

            def emit_P(n, h, st):
                """position matmuls + drains + scratch write + diag read"""
                qv = st["proj"][1]
                rel_h = rel_sb[(h % 2) * 64:(h % 2) * 64 + 64, h // 2, :]
                pd4 = pdpool.tile([128, 4, 640], BF16, tag="pd4")
                for qt in range(4):
                    ws = 384 - 128 * qt
                    P = pp.tile([128, 640], F32, tag="pp")
                    nc.tensor.matmul(P[:, 0:512], head_slice(qv, h, qt),
                                     rel_h[:, ws:ws + 512], start=True, stop=True)
                    nc.tensor.matmul(P[:, 512:640], head_slice(qv, h, qt),
                                     rel_h[:, ws + 512:ws + 640],
                                     start=True, stop=True)
                    if qt % 2 == 0:
                        nc.scalar.activation(pd4[:, qt, :], P[:], AF.Identity)
                    else:
                        nc.vector.tensor_copy(pd4[:, qt, :], P[:])
                scr = dscratch.tile([4, 128, 640], BF16, tag="scr")
                # write [p, qt, j] -> [qt, p, j]
                nc.sync.dma_start(
                    out=scr[:].rearrange("qt p j -> p qt j"), in_=pd4[:])
                # diagonal read: pos[p, qt, k] = scr[qt, p, 127 - p + k]
                pos4 = pospool.tile([128, 4, 512], BF16, tag="pos4")
                dsrc = bass.AP(scr[:].tensor, 127,
                               [[639, 128], [128 * 640, 4], [1, 512]])
                nc.sync.dma_start(out=pos4[:], in_=dsrc)
                st["pos4"] = pos4

            def emit_C(n, h, st):
                qu, _, kt, _ = st["proj"]
                st["Z"] = small.tile([128, 4], F32, tag="Z", name="zt")
                st["expm"] = hdat.tile([128, 4, 512], BF16, tag="expm",
                                       name="expm")
                for qt in range(4):
                    C = cc.tile([128, 512], F32, tag="cc")
                    nc.tensor.matmul(C[:], head_slice(qu, h, qt),
                                     kt[(h % 2) * 64:(h % 2) * 64 + 64, h // 2, :],
                                     start=True, stop=False)
                    if masked:
                        nc.tensor.matmul(C[:], ident[:], maskb[:, 4 * n + qt, :],
                                         start=False, stop=False)
                    nc.tensor.matmul(C[:], ident[:], st["pos4"][:, qt, :],
                                     start=False, stop=True)
                    nc.scalar.activation(st["expm"][:, qt, :], C[:], AF.Exp,
                                         scale=0.125,
                                         accum_out=st["Z"][:, qt:qt + 1])


            def emit_F1(n, h, st):
                rc = small.tile([128, 4], F32, tag="rc")
                nc.vector.reciprocal(rc[:], st["Z"][:])
                attn_n = st["expm"]
                for qt in range(4):
                    nc.vector.tensor_scalar(
                        out=attn_n[:, qt, :], in0=st["expm"][:, qt, :],
                        scalar1=rc[:, qt:qt + 1], scalar2=None, op0=ALU.mult)
                nc.gpsimd.dma_start(
                    out=attn_o[n, h].rearrange("(qt p) k -> p qt k", p=128),
                    in_=attn_n[:])

            def emit_F2(n, h, st):
                attn_t = hdat.tile([128, 4, 512], BF16, tag="attn_t")
                nc.scalar.dma_start_transpose(out=attn_t[:], in_=attn_o[n, h][:])
                vsb = st["proj"][3]
                ctxp = mm.tile([64, 512], F32, tag="mmt")
                for ktile in range(4):
                    nc.tensor.matmul(
                        ctxp[:],
                        vsb[:, ktile, 64 * h:64 * h + 64],
                        attn_t[:, ktile, :],
                        start=(ktile == 0), stop=(ktile == 3))
                nc.vector.tensor_copy(
                    st["ctx"][(h % 2) * 64:(h % 2) * 64 + 64, h // 2, :],
                    ctxp[:])

            def emit_E(n, st):
                osb = outp.tile([128, 4, 512], F32, tag="osb")
                for tt in range(4):
                    op = mm.tile([128, 512], F32, tag="mmt")
                    for et in range(4):
                        nc.tensor.matmul(
                            op[:], st["ctx"][:, et, 128 * tt:128 * tt + 128],
                            wot[:, et, :], start=(et == 0), stop=False)
                    nc.tensor.matmul(op[:], onescol[:], bo_b[:],
                                     start=False, stop=True)
                    nc.vector.tensor_copy(osb[:, tt, :], op[:])
                nc.gpsimd.dma_start(
                    out=out_o[n].rearrange("(tt p) e -> p tt e", p=128),
                    in_=osb[:])

            # ---- main stream over head PAIRS (dense PE bursts for HAM) ----
            PSKEW = 3  # pairs between P and C phases
            DF1 = 0    # pairs between C and F1
            DF2 = 2    # pairs between C and F2
            pairs = [(n, h) for n in range(NL) for h in range(0, HEADS, 2)]
            state = {}
            f1q, f2q = [], []
            cur_n = -1
            wot_done = [False]

            def do_F2(fn, fh, st):
                emit_F2(fn, fh, st)
                if fh == HEADS - 1:
                    emit_E(fn, state[(fn, fh)])

            def advance(do_c=None):
                if do_c is not None:
                    jn, jh = do_c
                    emit_C(jn, jh, state[(jn, jh)])
                    emit_C(jn, jh + 1, state[(jn, jh + 1)])
                    f1q.append((jn, jh))
                    f2q.append((jn, jh))
                while len(f1q) > DF1:
                    fn, fh = f1q.pop(0)
                    emit_F1(fn, fh, state[(fn, fh)])
                    emit_F1(fn, fh + 1, state[(fn, fh + 1)])
                while len(f2q) > DF2:
                    fn, fh = f2q.pop(0)
                    do_F2(fn, fh, state[(fn, fh)])
                    do_F2(fn, fh + 1, state[(fn, fh + 1)])

            for i, (n, h) in enumerate(pairs):
                if n != cur_n:
                    cur_n = n
                    xt = emit_slabs(n)
                    projt = emit_proj(n, xt)
                    ctx_sb = proj.tile([128, HP, QL], BF16, tag="ctx",
                                       name="ctx_sb")
                    nstate = {"proj": projt, "ctx": ctx_sb}
                for hh in (h, h + 1):
                    state[(n, hh)] = dict(nstate)
                    emit_P(n, hh, state[(n, hh)])
                if i == 1 and not wot_done[0]:
                    wot_done[0] = True
                    emit_wot()
                j = i - PSKEW
                if j >= 0:
                    advance(do_c=pairs[j])
            for j in range(len(pairs) - PSKEW, len(pairs)):
                advance(do_c=pairs[j])
            while f1q:
                fn, fh = f1q.pop(0)
                emit_F1(fn, fh, state[(fn, fh)])
                emit_F1(fn, fh + 1, state[(fn, fh + 1)])
            while f2q:
                fn, fh = f2q.pop(0)
                do_F2(fn, fh, state[(fn, fh)])
                do_F2(fn, fh + 1, state[(fn, fh + 1)])

    nc.compile()
    return nc


def _host_constants():
    invf = (10000.0 ** (-np.arange(0, EMBED, 2, dtype=np.float64) / EMBED))
    seq = np.arange(R, dtype=np.float64)[:, None] * invf[None, :]
    pos_emb = np.concatenate([np.sin(seq), np.cos(seq)], axis=-1)  # [R, E]
    post_rev = np.ascontiguousarray(pos_emb[::-1, :].T).astype(ml_dtypes.bfloat16)
    ident = np.eye(128, dtype=ml_dtypes.bfloat16)
    onescol = np.ones((1, 128), dtype=ml_dtypes.bfloat16)
    return post_rev, ident, onescol


def kernel(values, keys, query, mask, Wv, Wk, Wq, Wr, u_bias, v_bias, Wo, bo):
    values = np.ascontiguousarray(np.asarray(values, dtype=np.float32))
    keys = np.ascontiguousarray(np.asarray(keys, dtype=np.float32))
    query = np.ascontiguousarray(np.asarray(query, dtype=np.float32))
    mask = np.ascontiguousarray(np.asarray(mask, dtype=np.int32))
    Wv = np.ascontiguousarray(np.asarray(Wv, dtype=np.float32))
    Wk = np.ascontiguousarray(np.asarray(Wk, dtype=np.float32))
    Wq = np.ascontiguousarray(np.asarray(Wq, dtype=np.float32))
    Wr = np.ascontiguousarray(np.asarray(Wr, dtype=np.float32))
    Wo = np.ascontiguousarray(np.asarray(Wo, dtype=np.float32))
    u_flat = np.ascontiguousarray(np.asarray(u_bias, dtype=np.float32).reshape(1, EMBED))
    v_flat = np.ascontiguousarray(np.asarray(v_bias, dtype=np.float32).reshape(1, EMBED))
    bo_flat = np.ascontiguousarray(np.asarray(bo, dtype=np.float32).reshape(1, EMBED))

    post_rev, ident, onescol = _host_constants()

    masked = bool((mask == 0).any())
    nc = build_program(masked=masked)

    in_maps = []
    for c in range(NCORES):
        s = slice(NL * c, NL * (c + 1))
        in_maps.append({
            "q_in": query[s], "k_in": keys[s], "v_in": values[s],
            **({"m_in": mask[s]} if masked else {}),
            "wq_in": Wq, "wk_in": Wk, "wv_in": Wv, "wr_in": Wr, "wo_in": Wo,
            "u_in": u_flat, "vb_in": v_flat, "bo_in": bo_flat,
            "post_in": post_rev, "id_in": ident, "onescol_in": onescol,
        })

    res = run_bass_kernel_spmd(nc, in_maps, core_ids=list(range(NCORES)))
    global LAST_RESULTS
    LAST_RESULTS = res

    out = np.empty((NB, QL, EMBED), dtype=np.float32)
    attn = np.empty((NB, HEADS, QL, KL), dtype=np.float32)
    for c in range(NCORES):
        r = res.results[c]
        out[NL * c:NL * (c + 1)] = r["out_o"]
        attn[NL * c:NL * (c + 1)] = np.asarray(r["attn_o"], dtype=np.float32)
    return out, attn
